# revision 39
# baseline (speedup 1.0000x reference)
"""Causal self-attention (B=2, S=4096, D=512, H=8) on 8 Trainium2 cores.

Sharding: core c handles batch b = c//4 and heads {2*(c%4), 2*(c%4)+1}.

Fused single-pipeline design: per query-chunk J the kernel runs attention
for head0 then head1 (k-major transposed scores, exp on ACT with the
padding mask folded into the per-partition bias), while the PE slack under
the ACT-bound steady state absorbs interleaved "filler" work: q/k/v
projections for chunk J+1, V transposes, and the q-major output projection
whose units self-append as soon as their half of the PV accumulator is
final.  Scores PSUM is double-buffered and QK for kb+1 issues before PV
for kb so ACT never waits; the first QK of the next head is prefetched
before the current head's PV tail.  Outputs are per-head undivided
projections po_h [S, 512] bf16 plus softmax denominators; the host
divides, sums heads/cores, and adds bo.

PSUM map (8 banks): st 2 bufs x [128,1024]f32 (4) | pv [65,1024]f32 (2)
| aux 2 bufs x [128,512]f32 shared by proj pieces / V transposes / outproj.

Head x row-group layout: qT/kT keep head0 on partitions 0-63, head1 on
64-127; dupq/dupk hold the swapped copy so head h can issue even kb blocks
on PE row group 0 and odd kb blocks on row group 64 (concurrent tiles).
"""

import sys

sys.path.insert(0, "/opt/trn_rl_repo")

from contextlib import ExitStack

import ml_dtypes
import numpy as np

import concourse.bass as bass
import concourse.tile as tile
from concourse import bacc, bass_utils, mybir

B, S, D = 2, 4096, 512
H, HD = 8, 64
NCORES = 8
F32 = mybir.dt.float32
BF16 = mybir.dt.bfloat16
EXP = mybir.ActivationFunctionType.Exp
NPBF16 = ml_dtypes.bfloat16

CHUNK = 1024                  # query-chunk width
NCHUNK = S // CHUNK           # 4
KBLK = 128                    # key block (partition dim)
KB_PER_CHUNK = CHUNK // KBLK  # 8
NEG = -1.0e30


def _pieces(col0):
    """Split [col0, CHUNK) into <=512-wide pieces aligned to 512 boundaries."""
    out = []
    c = col0
    while c < CHUNK:
        nxt = min(CHUNK, (c // 512 + 1) * 512)
        out.append((c, nxt))
        c = nxt
    return out


class _Emitter:
    def __init__(self, nc, tc, ctx, io):
        self.nc = nc
        (self.xT, self.wq_p, self.wk_p, self.wv_p, self.wo01, self.bqkv,
         self.kbias, self.trimask, self.ident2, self.po0, self.po1,
         self.dens) = io

        const = ctx.enter_context(tc.tile_pool(name="const", bufs=1))
        self.sb = ctx.enter_context(tc.tile_pool(name="sb", bufs=1))
        self.etp = ctx.enter_context(tc.tile_pool(name="etp", bufs=8))
        self.xp = ctx.enter_context(tc.tile_pool(name="xp", bufs=2))
        self.ps_st = ctx.enter_context(
            tc.tile_pool(name="ps_st", bufs=2, space="PSUM"))
        self.ps_pv = ctx.enter_context(
            tc.tile_pool(name="ps_pv", bufs=1, space="PSUM"))
        self.ps_aux = ctx.enter_context(
            tc.tile_pool(name="ps_aux", bufs=2, space="PSUM"))

        # constants / weights
        self.wq_sb = const.tile([128, 512], BF16, tag="wq")
        self.wk_sb = const.tile([128, 512], BF16, tag="wk")
        self.wv_sb = const.tile([128, 512], BF16, tag="wv")
        self.wo_sb = const.tile([128, 512], BF16, tag="wo")
        self.bqkv_sb = const.tile([128, 3], F32, tag="bqkv")
        self.kbias_sb = const.tile([128, 32], F32, tag="kbias")
        self.tri_sb = const.tile([128, 128], BF16, tag="tri")
        self.id2_sb = const.tile([128, 64], BF16, tag="id2")
        onesf_sb = const.tile([128, 1], F32, tag="onesf")
        nc.vector.memset(onesf_sb[:], 1.0)

        # persistent intermediates
        self.qT = self.sb.tile([128, S], BF16, tag="qT")
        self.kT = self.sb.tile([128, S], BF16, tag="kT")
        self.dupq = self.sb.tile([128, S], BF16, tag="dupq")
        self.dupk = self.sb.tile([128, S], BF16, tag="dupk")
        self.v0 = self.sb.tile([128, 32 * 65], BF16, tag="v0")
        self.v1 = self.sb.tile([128, 32 * 65], BF16, tag="v1")
        self.oT01 = self.sb.tile([128, S], BF16, tag="oT01")
        self.den0 = self.sb.tile([1, S], F32, tag="den0")
        self.den1 = self.sb.tile([1, S], F32, tag="den1")
        for vdst in (self.v0, self.v1):
            ones_col = vdst.rearrange("p (k c) -> p k c", c=65)[:, :, 64:65]
            nc.vector.tensor_copy(
                ones_col, onesf_sb[:].to_broadcast((128, 32, 1)))

        self._fill = []
        self._calls_left = 1

    # ---------------- filler machinery -----------------------------------
    def fill(self):
        for _ in range(self._rate):
            if self._fill:
                self._fill.pop(0)()

    def flush_fill(self):
        while self._fill:
            self._fill.pop(0)()

    def emit_dma_x(self, J):
        """Queue DMA of x chunk J (two 512-col halves). Returns x tiles."""
        nc = self.nc
        x_sb = []
        for ks in range(4):
            xt = self.xp.tile([128, CHUNK], BF16, tag=f"x{ks}")
            x_sb.append(xt)

        def half_unit(half):
            lo = half * 512
            for ks in range(4):
                nc.sync.dma_start(
                    x_sb[ks][:, lo:lo + 512],
                    self.xT[ks * 128:(ks + 1) * 128,
                            J * CHUNK + lo:J * CHUNK + lo + 512])

        def unit():
            half_unit(0)
            half_unit(1)
        return x_sb, unit, half_unit

    def _mk_proj(self, x_sb, w_sb, bcol, dest, base, lo):
        """Two filler units (2 accum MMs each) sharing one aux slot, for
        finer interleave pacing. Returns [unit_a, unit_b]."""
        nc = self.nc
        cell = {}

        def unit_a():
            ps = self.ps_aux.tile([128, 512], F32, tag="aux", name="ps")
            cell["ps"] = ps
            for ks in range(2):
                nc.tensor.matmul(
                    ps[:],
                    w_sb[:, ks * 128:(ks + 1) * 128],
                    x_sb[ks][:, lo:lo + 512],
                    start=(ks == 0), stop=False)

        def unit_b():
            ps = cell["ps"]
            for ks in range(2, 4):
                nc.tensor.matmul(
                    ps[:],
                    w_sb[:, ks * 128:(ks + 1) * 128],
                    x_sb[ks][:, lo:lo + 512],
                    start=False, stop=(ks == 3))
            nc.vector.tensor_scalar_add(
                dest[:, base + lo:base + lo + 512], ps[:],
                self.bqkv_sb[:, bcol:bcol + 1])
        return [unit_a, unit_b]

    def proj_q_units(self, J, x_sb):
        """q projection + dupq swap for chunk J (needed at chunk start)."""
        nc = self.nc
        csl = slice(J * CHUNK, (J + 1) * CHUNK)
        units = []
        for lo in (0, 512):
            units += self._mk_proj(x_sb, self.wq_sb, 0, self.qT,
                                   J * CHUNK, lo)

        def dupq_unit():
            nc.vector.tensor_copy(self.dupq[64:128, csl], self.qT[0:64, csl])
            nc.vector.tensor_copy(self.dupq[0:64, csl], self.qT[64:128, csl])
        units.append(dupq_unit)
        return units

    def proj_kv_units(self, J, x_sb):
        """k/v projections + dupk + V transpose/repack for chunk J (needed
        only once attention reaches chunk J's diagonal blocks)."""
        nc = self.nc
        csl = slice(J * CHUNK, (J + 1) * CHUNK)
        vT = self.sb.tile([128, CHUNK], BF16, tag="vTc", bufs=2)
        units = []
        for lo in (0, 512):
            units += self._mk_proj(x_sb, self.wk_sb, 1, self.kT,
                                   J * CHUNK, lo)

        def dupk_unit():
            nc.vector.tensor_copy(self.dupk[64:128, csl], self.kT[0:64, csl])
            nc.vector.tensor_copy(self.dupk[0:64, csl], self.kT[64:128, csl])
        units.append(dupk_unit)

        for lo in (0, 512):
            units += self._mk_proj(x_sb, self.wv_sb, 2, vT, 0, lo)

        # V -> k-major 65-stride blocks (per head)
        for hh, vdst in ((0, self.v0), (1, self.v1)):
            def unit(hh=hh, vdst=vdst):
                tr = self.ps_aux.tile([128, 512], BF16, tag="aux")
                for i in range(8):
                    nc.tensor.transpose(
                        tr[:, i * 64:(i + 1) * 64],
                        vT[hh * 64:(hh + 1) * 64, i * KBLK:(i + 1) * KBLK],
                        self.id2_sb[hh * 64:(hh + 1) * 64, :])
                dst = vdst[:, J * 8 * 65:(J + 1) * 8 * 65]
                dst = dst.rearrange("p (k c) -> p k c", c=65)[:, :, 0:64]
                nc.vector.tensor_copy(
                    dst, tr.rearrange("p (k c) -> p k c", c=64))
            units.append(unit)
        return units

    def ph3_units(self, J, hh, qb0, qb1, scalar_cast=False):
        """Output projection units for (chunk J, head hh), qb in [qb0,qb1).

        scalar_cast routes the PSUM->SBUF cast to the ACT engine — only
        valid when no exp work remains (it would delay the exp chain)."""
        nc = self.nc
        units = []
        hsl = slice(hh * 64, (hh + 1) * 64)
        dram = self.po0 if hh == 0 else self.po1
        for qb in range(qb0, qb1):
            def unit(qb=qb):
                q0 = J * CHUNK + qb * 128
                po = self.ps_aux.tile([128, 512], F32, tag="aux")
                nc.tensor.matmul(
                    po[:], self.oT01[hsl, q0:q0 + 128],
                    self.wo_sb[hsl, :], start=True, stop=True)
                posb = self.sb.tile([128, 512], BF16, tag="posb", bufs=4)
                if scalar_cast:
                    nc.scalar.copy(posb[:], po[:])
                else:
                    nc.vector.tensor_copy(posb[:], po[:])
                nc.sync.dma_start(dram[q0:q0 + 128, :], posb[:])
            units.append(unit)
        return units

    # ---------------- attention ------------------------------------------
    def _emit_qk(self, J, hh, kb):
        nc = self.nc
        p = kb - KB_PER_CHUNK * J
        col0 = KBLK * p if p >= 0 else 0
        # even kb -> natural layout at the head's home group; odd kb ->
        # swapped dup layout at the other group (concurrent PE tiles)
        if kb % 2 == 0:
            qsrc, ksrc, g = self.qT, self.kT, hh * 64
        else:
            qsrc, ksrc, g = self.dupq, self.dupk, 64 - hh * 64
        gs = slice(g, g + 64)
        st = self.ps_st.tile([128, CHUNK], F32, tag="st")
        for (a, b) in _pieces(col0):
            nc.tensor.matmul(
                st[:, a:b],
                ksrc[gs, kb * KBLK:(kb + 1) * KBLK],
                qsrc[gs, J * CHUNK + a:J * CHUNK + b],
                start=True, stop=True)
        return st

    def attention(self, J, hh, st0=None, prefetch=None, drain_early=False):
        """Attention for (chunk J, head hh). Returns the prefetched st of
        `prefetch` = (J', hh') if given (emitted before our PV tail)."""
        nc = self.nc
        vsb = self.v0 if hh == 0 else self.v1
        den = self.den0 if hh == 0 else self.den1
        hsl = slice(hh * 64, (hh + 1) * 64)
        pv = self.ps_pv.tile([65, CHUNK], F32, tag="pv")
        nkb = KB_PER_CHUNK * (J + 1)
        c0 = J * CHUNK
        st_next = None

        def emit_pv(kb, et, pieces):
            for (a, b) in pieces:
                last_a = (kb == KB_PER_CHUNK * J + 3 and a < 512)
                last_b = (kb == nkb - 1)
                nc.tensor.matmul(
                    pv[:, a:b],
                    vsb[:, kb * 65:(kb + 1) * 65],
                    et[:, a:b],
                    start=(kb == 0),
                    stop=(last_a if a < 512 else last_b))
            if drain_early and kb == KB_PER_CHUNK * J + 3:
                # pv[:, 0:512] final: drain it so its outproj can overlap
                nc.vector.tensor_copy(
                    self.oT01[hsl, c0:c0 + 512], pv[0:64, 0:512])
                nc.vector.tensor_copy(den[:, c0:c0 + 512], pv[64:65, 0:512])
                self._fill += self.ph3_units(J, hh, 0, 4)

        # software pipeline: PV lags one block so its et/tri deps are
        # already satisfied when the in-order PE queue reaches it.
        lagged = None
        st = st0 if st0 is not None else self._emit_qk(J, hh, 0)
        for kb in range(nkb):
            p = kb - KB_PER_CHUNK * J
            col0 = KBLK * p if p >= 0 else 0
            pieces = _pieces(col0)
            et = self.etp.tile([128, CHUNK], BF16, tag="et")
            nc.scalar.activation(
                et[:, col0:], st[:, col0:], EXP,
                bias=self.kbias_sb[:, kb:kb + 1], scale=0.125)
            if p >= 0:
                nc.vector.tensor_mul(
                    et[:, col0:col0 + KBLK], et[:, col0:col0 + KBLK],
                    self.tri_sb[:])
            if kb + 1 < nkb:
                st = self._emit_qk(J, hh, kb + 1)
            elif prefetch is not None:
                st_next = self._emit_qk(prefetch[0], prefetch[1], 0)
            self.fill()
            if lagged is not None:
                emit_pv(*lagged)
            lagged = (kb, et, pieces)
        emit_pv(*lagged)
        if drain_early:
            # the exp chain is finished: use the idle ACT engine for the
            # tail drain so the DVE (trimask/casts) isn't the serializer
            nc.scalar.copy(self.oT01[hsl, c0 + 512:c0 + CHUNK],
                           pv[0:64, 512:])
            nc.scalar.copy(den[:, c0 + 512:c0 + CHUNK], pv[64:65, 512:])
            self._fill += self.ph3_units(J, hh, 4, 8, scalar_cast=True)
        else:
            nc.vector.tensor_copy(self.oT01[hsl, c0:c0 + CHUNK], pv[0:64, :])
            nc.vector.tensor_copy(den[:, c0:c0 + CHUNK], pv[64:65, :])
        return st_next

    def run(self):
        nc = self.nc
        # prologue: x chunk-0 half0 + q/k weights land first so the first
        # projection starts as early as possible; remaining consts follow.
        x0, _, x0_half = self.emit_dma_x(0)
        x0_half(0)
        for t, a in ((self.wq_sb, self.wq_p), (self.bqkv_sb, self.bqkv),
                     (self.wk_sb, self.wk_p)):
            nc.sync.dma_start(t[:], a[:])
        x0_half(1)
        for t, a in ((self.wv_sb, self.wv_p), (self.id2_sb, self.ident2),
                     (self.kbias_sb, self.kbias), (self.tri_sb, self.trimask),
                     (self.wo_sb, self.wo01)):
            nc.sync.dma_start(t[:], a[:])
        # preload the exp table with a dummy 1-column activation so the
        # first real exp doesn't pay the ACT_TABLE_LOAD
        scratch = self.sb.tile([128, 1], F32, tag="scratch")
        nc.scalar.activation(scratch[:], self.bqkv_sb[:, 0:1], EXP,
                             bias=0.0, scale=0.0)
        # only what attention(0, h0) needs up front (q/k/dups); chunk 0's
        # v path runs as rate-4 fillers at the first iteration.  The first
        # QK issues as soon as q (full) and k half0 (covers key block 0)
        # exist; k half1 and the dup copies follow.
        q0u = self.proj_q_units(0, x0)
        kv0 = self.proj_kv_units(0, x0)
        for u in (q0u[0], q0u[1], kv0[0], q0u[2], q0u[3], kv0[1]):
            u()  # q h0 (2), k h0 a, q h1 (2), k h0 b
        st_carry = self._emit_qk(0, 0, 0)
        kv0[2]()  # k h1 a
        kv0[3]()  # k h1 b
        q0u[4]()  # dupq
        kv0[4]()  # dupk
        x1, dma1, _ = self.emit_dma_x(1)
        dma1()

        xs = {1: x1}
        for J in range(NCHUNK):
            nkb = KB_PER_CHUNK * (J + 1)
            # head0 fills: this chunk's k/v tail work (J=0: v path only),
            # next x DMA, and the previous chunk's head1 output projection.
            fills_h0 = kv0[5:] if J == 0 else self.proj_kv_units(J, xs[J])
            if 2 <= J + 1 < NCHUNK:  # chunk 1 was DMA'd in the prologue
                xn, dman, _ = self.emit_dma_x(J + 1)
                xs[J + 1] = xn
                fills_h0 = fills_h0 + [dman]
            if J > 0:
                # chunk 1 is the tightest (16 fills over 16 iters): push
                # half of the previous outproj into the head1 window
                hi = 4 if J == 1 else 8
                fills_h0 = fills_h0 + self.ph3_units(J - 1, 1, 0, hi)
            # head1 fills: next chunk's q units + this chunk's h0 outproj
            fills_h1 = []
            if J + 1 < NCHUNK:
                fills_h1 += self.proj_q_units(J + 1, xs[J + 1])
            if J == 1:
                fills_h1 += self.ph3_units(0, 1, 4, 8)
            fills_h1 += self.ph3_units(J, 0, 0, 8)

            self._fill = self._fill + fills_h0
            self._rate = 4 if J == 0 else max(
                1, (len(self._fill) + nkb - 1) // nkb)
            st_carry = self.attention(J, 0, st0=st_carry, prefetch=(J, 1))

            self._fill = self._fill + fills_h1
            self._rate = max(1, (len(self._fill) + nkb - 1) // nkb)
            pf = (J + 1, 0) if J + 1 < NCHUNK else None
            st_carry = self.attention(J, 1, st0=st_carry, prefetch=pf,
                                      drain_early=(J == NCHUNK - 1))

        self.flush_fill()
        nc.sync.dma_start(self.dens[0:1, :], self.den0[:])
        nc.sync.dma_start(self.dens[1:2, :], self.den1[:])


def _emit(nc, tc, ctx, io):
    _Emitter(nc, tc, ctx, io).run()


_CACHED = None


def _build():
    global _CACHED
    if _CACHED is not None:
        return _CACHED
    nc = bacc.Bacc("TRN2", target_bir_lowering=False, debug=False,
                   enable_asserts=False, num_devices=NCORES)
    names = [
        ("xT", [D, S], BF16), ("wq_p", [128, 512], BF16),
        ("wk_p", [128, 512], BF16), ("wv_p", [128, 512], BF16),
        ("wo01", [128, 512], BF16),
        ("bqkv", [128, 3], F32), ("kbias", [128, 32], F32),
        ("trimask", [128, 128], BF16), ("ident2", [128, 64], BF16),
    ]
    aps = [nc.dram_tensor(n, sh, dt_, kind="ExternalInput").ap()
           for n, sh, dt_ in names]
    po0 = nc.dram_tensor("po0", [S, D], BF16, kind="ExternalOutput").ap()
    po1 = nc.dram_tensor("po1", [S, D], BF16, kind="ExternalOutput").ap()
    dens = nc.dram_tensor("dens", [2, S], F32, kind="ExternalOutput").ap()
    with tile.TileContext(nc) as tc, ExitStack() as ctx:
        _emit(nc, tc, ctx, aps + [po0, po1, dens])
    nc.compile()
    _CACHED = nc
    return nc


def _host_inputs(x, attention_mask, Wq, bq, Wk, bk, Wv, bv, Wo, bo):
    f = np.float32
    x = np.asarray(x, f)
    mask = np.asarray(attention_mask)
    Wq, Wk, Wv, Wo = (np.asarray(w, f) for w in (Wq, Wk, Wv, Wo))
    bq, bk, bv = (np.asarray(b_, f) for b_ in (bq, bk, bv))
    tri = np.triu(np.ones((128, 128), NPBF16))      # [k,q]: 1 where q >= k
    id2 = np.tile(np.eye(64, dtype=NPBF16), (2, 1))
    in_maps = []
    for c in range(NCORES):
        b = c // 4
        h0 = 2 * (c % 4)
        hsl = slice(64 * h0, 64 * h0 + 128)

        def pack_w(W):
            wt = W[hsl, :].T                        # [512, 128] = Wh^T
            return np.ascontiguousarray(
                wt.reshape(4, 128, 128).transpose(1, 0, 2)
                .reshape(128, 512).astype(NPBF16))

        wo_t = Wo[:, hsl].T.astype(NPBF16)           # [128, 512]
        kb = np.where(mask[b] != 0, f(0.0), f(NEG)).astype(f)
        in_maps.append({
            "xT": np.ascontiguousarray(x[b].T.astype(NPBF16)),
            "wq_p": pack_w(Wq), "wk_p": pack_w(Wk), "wv_p": pack_w(Wv),
            "wo01": np.ascontiguousarray(wo_t),
            "bqkv": np.ascontiguousarray(
                np.stack([bq[hsl], bk[hsl], bv[hsl]], axis=1)),
            "kbias": np.ascontiguousarray(kb.reshape(32, 128).T),
            "trimask": tri, "ident2": id2,
        })
    return in_maps


def _assemble(results, bo):
    out = np.zeros((B, S, D), np.float32)
    for c in range(NCORES):
        r = results[c]
        dens = r["dens"]
        part = (r["po0"].astype(np.float32) / dens[0][:, None]
                + r["po1"].astype(np.float32) / dens[1][:, None])
        out[c // 4] += part
    out += np.asarray(bo, np.float32)
    return out


def kernel(**inputs) -> np.ndarray:
    nc = _build()
    in_maps = _host_inputs(**inputs)
    last_err = None
    for attempt in range(3):
        try:
            res = bass_utils.run_bass_kernel_spmd(
                nc, in_maps, core_ids=list(range(NCORES)))
            out = _assemble(res.results, inputs["bo"])
        except Exception as e:  # transient NRT/axon device errors
            last_err = e
            continue
        if np.isfinite(out).all():
            return out
        last_err = RuntimeError("non-finite output")
    raise last_err


def run_traced(inputs, **kwargs):
    """test.py helper: run with NTFF tracing, return (out, BassKernelResults)."""
    nc = _build()
    in_maps = _host_inputs(**inputs)
    res = bass_utils.run_bass_kernel_spmd(
        nc, in_maps, core_ids=list(range(NCORES)), trace=True, **kwargs)
    return _assemble(res.results, inputs["bo"]), res


# revision 42
# speedup vs baseline: 1.1953x; 1.1953x over previous
"""Causal self-attention (B=2, S=4096, D=512, H=8) on 8 Trainium2 cores.

Sharding: core c handles batch b = c//4 and heads {2*(c%4), 2*(c%4)+1}.

Fused single-pipeline design: per query-chunk J the kernel runs attention
for head0 then head1 (k-major transposed scores, exp on ACT with the
padding mask folded into the per-partition bias), while the PE slack under
the ACT-bound steady state absorbs interleaved "filler" work: q/k/v
projections for chunk J+1, V transposes, and the q-major output projection
whose units self-append as soon as their half of the PV accumulator is
final.  Scores PSUM is double-buffered and QK for kb+1 issues before PV
for kb so ACT never waits; the first QK of the next head is prefetched
before the current head's PV tail.  Outputs are per-head undivided
projections po_h [S, 512] bf16 plus softmax denominators; the host
divides, sums heads/cores, and adds bo.

PSUM map (8 banks): st 2 bufs x [128,1024]f32 (4) | pv [65,1024]f32 (2)
| aux 2 bufs x [128,512]f32 shared by proj pieces / V transposes / outproj.

Head x row-group layout: qT/kT keep head0 on partitions 0-63, head1 on
64-127; dupq/dupk hold the swapped copy so head h can issue even kb blocks
on PE row group 0 and odd kb blocks on row group 64 (concurrent tiles).
"""

import sys

sys.path.insert(0, "/opt/trn_rl_repo")

from contextlib import ExitStack

import ml_dtypes
import numpy as np

import concourse.bass as bass
import concourse.tile as tile
from concourse import bacc, bass_utils, mybir

B, S, D = 2, 4096, 512
H, HD = 8, 64
NCORES = 8
F32 = mybir.dt.float32
BF16 = mybir.dt.bfloat16
EXP = mybir.ActivationFunctionType.Exp
NPBF16 = ml_dtypes.bfloat16

CHUNK = 1024                  # query-chunk width
NCHUNK = S // CHUNK           # 4
KBLK = 128                    # key block (partition dim)
KB_PER_CHUNK = CHUNK // KBLK  # 8
NEG = -1.0e30


def _pieces(col0):
    """Split [col0, CHUNK) into <=512-wide pieces aligned to 512 boundaries."""
    out = []
    c = col0
    while c < CHUNK:
        nxt = min(CHUNK, (c // 512 + 1) * 512)
        out.append((c, nxt))
        c = nxt
    return out


class _Emitter:
    def __init__(self, nc, tc, ctx, io):
        self.nc = nc
        (self.xT, self.wq_p, self.wk_p, self.wv_p, self.wo01, self.bqkv,
         self.kbias, self.trimask, self.ident2, self.po0, self.po1,
         self.dens) = io

        const = ctx.enter_context(tc.tile_pool(name="const", bufs=1))
        self.sb = ctx.enter_context(tc.tile_pool(name="sb", bufs=1))
        self.etp = ctx.enter_context(tc.tile_pool(name="etp", bufs=8))
        self.xp = ctx.enter_context(tc.tile_pool(name="xp", bufs=2))
        self.ps_st = ctx.enter_context(
            tc.tile_pool(name="ps_st", bufs=2, space="PSUM"))
        self.ps_pv = ctx.enter_context(
            tc.tile_pool(name="ps_pv", bufs=1, space="PSUM"))
        self.ps_aux = ctx.enter_context(
            tc.tile_pool(name="ps_aux", bufs=2, space="PSUM"))

        # constants / weights
        self.wq_sb = const.tile([128, 512], BF16, tag="wq")
        self.wk_sb = const.tile([128, 512], BF16, tag="wk")
        self.wv_sb = const.tile([128, 512], BF16, tag="wv")
        self.wo_sb = const.tile([128, 512], BF16, tag="wo")
        self.bqkv_sb = const.tile([128, 3], F32, tag="bqkv")
        self.kbias_sb = const.tile([128, 32], F32, tag="kbias")
        self.tri_sb = const.tile([128, 128], BF16, tag="tri")
        self.id2_sb = const.tile([128, 64], BF16, tag="id2")
        onesf_sb = const.tile([128, 1], F32, tag="onesf")
        nc.vector.memset(onesf_sb[:], 1.0)

        # persistent intermediates
        self.qT = self.sb.tile([128, S], BF16, tag="qT")
        self.kT = self.sb.tile([128, S], BF16, tag="kT")
        self.dupq = self.sb.tile([128, S], BF16, tag="dupq")
        self.dupk = self.sb.tile([128, S], BF16, tag="dupk")
        self.v0 = self.sb.tile([128, 32 * 65], BF16, tag="v0")
        self.v1 = self.sb.tile([128, 32 * 65], BF16, tag="v1")
        self.oT01 = self.sb.tile([128, S], BF16, tag="oT01")
        self.den0 = self.sb.tile([1, S], F32, tag="den0")
        self.den1 = self.sb.tile([1, S], F32, tag="den1")
        for vdst in (self.v0, self.v1):
            ones_col = vdst.rearrange("p (k c) -> p k c", c=65)[:, :, 64:65]
            nc.vector.tensor_copy(
                ones_col, onesf_sb[:].to_broadcast((128, 32, 1)))

        self._fill = []
        self._calls_left = 1

    # ---------------- filler machinery -----------------------------------
    def fill(self):
        for _ in range(self._rate):
            if self._fill:
                self._fill.pop(0)()

    def flush_fill(self):
        while self._fill:
            self._fill.pop(0)()

    def emit_dma_x(self, J):
        """Queue DMA of x chunk J (two 512-col halves). Returns x tiles."""
        nc = self.nc
        x_sb = []
        for ks in range(4):
            xt = self.xp.tile([128, CHUNK], BF16, tag=f"x{ks}")
            x_sb.append(xt)

        def half_unit(half):
            lo = half * 512
            for ks in range(4):
                nc.sync.dma_start(
                    x_sb[ks][:, lo:lo + 512],
                    self.xT[ks * 128:(ks + 1) * 128,
                            J * CHUNK + lo:J * CHUNK + lo + 512])

        def unit():
            half_unit(0)
            half_unit(1)
        return x_sb, unit, half_unit

    def _mk_proj(self, x_sb, w_sb, bcol, dest, base, lo):
        nc = self.nc

        def unit():
            ps = self.ps_aux.tile([128, 512], F32, tag="aux", name="ps")
            for ks in range(4):
                nc.tensor.matmul(
                    ps[:],
                    w_sb[:, ks * 128:(ks + 1) * 128],
                    x_sb[ks][:, lo:lo + 512],
                    start=(ks == 0), stop=(ks == 3))
            nc.vector.tensor_scalar_add(
                dest[:, base + lo:base + lo + 512], ps[:],
                self.bqkv_sb[:, bcol:bcol + 1])
        return [unit]

    def proj_q_units(self, J, x_sb):
        """q projection + dupq swap for chunk J (needed at chunk start)."""
        nc = self.nc
        csl = slice(J * CHUNK, (J + 1) * CHUNK)
        units = []
        for lo in (0, 512):
            units += self._mk_proj(x_sb, self.wq_sb, 0, self.qT,
                                   J * CHUNK, lo)

        def dupq_unit():
            nc.vector.tensor_copy(self.dupq[64:128, csl], self.qT[0:64, csl])
            nc.vector.tensor_copy(self.dupq[0:64, csl], self.qT[64:128, csl])
        units.append(dupq_unit)
        return units

    def proj_kv_units(self, J, x_sb):
        """k/v projections + dupk + V transpose/repack for chunk J (needed
        only once attention reaches chunk J's diagonal blocks)."""
        nc = self.nc
        csl = slice(J * CHUNK, (J + 1) * CHUNK)
        vT = self.sb.tile([128, CHUNK], BF16, tag="vTc", bufs=2)
        units = []
        for lo in (0, 512):
            units += self._mk_proj(x_sb, self.wk_sb, 1, self.kT,
                                   J * CHUNK, lo)

        def dupk_unit():
            nc.vector.tensor_copy(self.dupk[64:128, csl], self.kT[0:64, csl])
            nc.vector.tensor_copy(self.dupk[0:64, csl], self.kT[64:128, csl])
        units.append(dupk_unit)

        for lo in (0, 512):
            units += self._mk_proj(x_sb, self.wv_sb, 2, vT, 0, lo)

        # V -> k-major 65-stride blocks (per head)
        for hh, vdst in ((0, self.v0), (1, self.v1)):
            def unit(hh=hh, vdst=vdst):
                tr = self.ps_aux.tile([128, 512], BF16, tag="aux")
                for i in range(8):
                    nc.tensor.transpose(
                        tr[:, i * 64:(i + 1) * 64],
                        vT[hh * 64:(hh + 1) * 64, i * KBLK:(i + 1) * KBLK],
                        self.id2_sb[hh * 64:(hh + 1) * 64, :])
                dst = vdst[:, J * 8 * 65:(J + 1) * 8 * 65]
                dst = dst.rearrange("p (k c) -> p k c", c=65)[:, :, 0:64]
                nc.vector.tensor_copy(
                    dst, tr.rearrange("p (k c) -> p k c", c=64))
            units.append(unit)
        return units

    def ph3_units(self, J, hh, qb0, qb1, scalar_cast=False):
        """Output projection units for (chunk J, head hh), qb in [qb0,qb1).

        scalar_cast routes the PSUM->SBUF cast to the ACT engine — only
        valid when no exp work remains (it would delay the exp chain)."""
        nc = self.nc
        units = []
        hsl = slice(hh * 64, (hh + 1) * 64)
        dram = self.po0 if hh == 0 else self.po1
        for qb in range(qb0, qb1):
            def unit(qb=qb):
                q0 = J * CHUNK + qb * 128
                po = self.ps_aux.tile([128, 512], F32, tag="aux")
                nc.tensor.matmul(
                    po[:], self.oT01[hsl, q0:q0 + 128],
                    self.wo_sb[hsl, :], start=True, stop=True)
                posb = self.sb.tile([128, 512], BF16, tag="posb", bufs=4)
                if scalar_cast:
                    nc.scalar.copy(posb[:], po[:])
                else:
                    nc.vector.tensor_copy(posb[:], po[:])
                nc.sync.dma_start(dram[q0:q0 + 128, :], posb[:])
            units.append(unit)
        return units

    # ---------------- attention ------------------------------------------
    def _emit_qk(self, J, hh, kb):
        nc = self.nc
        p = kb - KB_PER_CHUNK * J
        col0 = KBLK * p if p >= 0 else 0
        # even kb -> natural layout at the head's home group; odd kb ->
        # swapped dup layout at the other group (concurrent PE tiles)
        if kb % 2 == 0:
            qsrc, ksrc, g = self.qT, self.kT, hh * 64
        else:
            qsrc, ksrc, g = self.dupq, self.dupk, 64 - hh * 64
        gs = slice(g, g + 64)
        st = self.ps_st.tile([128, CHUNK], F32, tag="st")
        for (a, b) in _pieces(col0):
            nc.tensor.matmul(
                st[:, a:b],
                ksrc[gs, kb * KBLK:(kb + 1) * KBLK],
                qsrc[gs, J * CHUNK + a:J * CHUNK + b],
                start=True, stop=True)
        return st

    def attention(self, J, hh, st0=None, prefetch=None, drain_early=False):
        """Attention for (chunk J, head hh). Returns the prefetched st of
        `prefetch` = (J', hh') if given (emitted before our PV tail)."""
        nc = self.nc
        vsb = self.v0 if hh == 0 else self.v1
        den = self.den0 if hh == 0 else self.den1
        hsl = slice(hh * 64, (hh + 1) * 64)
        pv = self.ps_pv.tile([65, CHUNK], F32, tag="pv")
        nkb = KB_PER_CHUNK * (J + 1)
        c0 = J * CHUNK
        st_next = None

        def emit_pv(kb, et, pieces):
            for (a, b) in pieces:
                last_a = (kb == KB_PER_CHUNK * J + 3 and a < 512)
                last_b = (kb == nkb - 1)
                nc.tensor.matmul(
                    pv[:, a:b],
                    vsb[:, kb * 65:(kb + 1) * 65],
                    et[:, a:b],
                    start=(kb == 0),
                    stop=(last_a if a < 512 else last_b))
            if drain_early and kb == KB_PER_CHUNK * J + 3:
                # pv[:, 0:512] final: drain it so its outproj can overlap
                nc.vector.tensor_copy(
                    self.oT01[hsl, c0:c0 + 512], pv[0:64, 0:512])
                nc.vector.tensor_copy(den[:, c0:c0 + 512], pv[64:65, 0:512])
                self._fill += self.ph3_units(J, hh, 0, 4)

        # software pipeline: PV lags one block so its et/tri deps are
        # already satisfied when the in-order PE queue reaches it.
        lagged = None
        st = st0 if st0 is not None else self._emit_qk(J, hh, 0)
        for kb in range(nkb):
            p = kb - KB_PER_CHUNK * J
            col0 = KBLK * p if p >= 0 else 0
            pieces = _pieces(col0)
            et = self.etp.tile([128, CHUNK], BF16, tag="et")
            nc.scalar.activation(
                et[:, col0:], st[:, col0:], EXP,
                bias=self.kbias_sb[:, kb:kb + 1], scale=0.125)
            if p >= 0:
                nc.vector.tensor_mul(
                    et[:, col0:col0 + KBLK], et[:, col0:col0 + KBLK],
                    self.tri_sb[:])
            if kb + 1 < nkb:
                st = self._emit_qk(J, hh, kb + 1)
            elif prefetch is not None:
                st_next = self._emit_qk(prefetch[0], prefetch[1], 0)
            self.fill()
            if lagged is not None:
                emit_pv(*lagged)
            lagged = (kb, et, pieces)
        emit_pv(*lagged)
        if drain_early:
            # the exp chain is finished: use the idle ACT engine for the
            # tail drain so the DVE (trimask/casts) isn't the serializer
            nc.scalar.copy(self.oT01[hsl, c0 + 512:c0 + CHUNK],
                           pv[0:64, 512:])
            nc.scalar.copy(den[:, c0 + 512:c0 + CHUNK], pv[64:65, 512:])
            self._fill += self.ph3_units(J, hh, 4, 8, scalar_cast=True)
        else:
            nc.vector.tensor_copy(self.oT01[hsl, c0:c0 + CHUNK], pv[0:64, :])
            nc.vector.tensor_copy(den[:, c0:c0 + CHUNK], pv[64:65, :])
        return st_next

    def run(self):
        nc = self.nc
        # prologue: x chunk-0 half0 + q/k weights land first so the first
        # projection starts as early as possible; remaining consts follow.
        x0, _, x0_half = self.emit_dma_x(0)
        x0_half(0)
        for t, a in ((self.wq_sb, self.wq_p), (self.bqkv_sb, self.bqkv),
                     (self.wk_sb, self.wk_p)):
            nc.sync.dma_start(t[:], a[:])
        x0_half(1)
        for t, a in ((self.wv_sb, self.wv_p), (self.id2_sb, self.ident2),
                     (self.kbias_sb, self.kbias), (self.tri_sb, self.trimask),
                     (self.wo_sb, self.wo01)):
            nc.sync.dma_start(t[:], a[:])
        # preload the exp table with a dummy 1-column activation so the
        # first real exp doesn't pay the ACT_TABLE_LOAD
        scratch = self.sb.tile([128, 1], F32, tag="scratch")
        nc.scalar.activation(scratch[:], self.bqkv_sb[:, 0:1], EXP,
                             bias=0.0, scale=0.0)
        # only what attention(0, h0) needs up front (q/k/dups); chunk 0's
        # v path runs as rate-4 fillers at the first iteration.  The first
        # QK issues as soon as q (full) and k half0 (covers key block 0)
        # exist; k half1 and the dup copies follow.
        q0u = self.proj_q_units(0, x0)
        kv0 = self.proj_kv_units(0, x0)
        for u in (q0u[0], q0u[1], kv0[0]):
            u()  # q half0, q half1, k half0
        st_carry = self._emit_qk(0, 0, 0)  # key block 0 needs only k half0
        kv0[1]()  # k half1
        q0u[2]()  # dupq
        kv0[2]()  # dupk
        x1, dma1, _ = self.emit_dma_x(1)
        dma1()

        xs = {1: x1}
        for J in range(NCHUNK):
            nkb = KB_PER_CHUNK * (J + 1)
            # head0 fills: this chunk's k/v tail work (J=0: v path only),
            # next x DMA, and the previous chunk's head1 output projection.
            fills_h0 = kv0[3:] if J == 0 else self.proj_kv_units(J, xs[J])
            if 2 <= J + 1 < NCHUNK:  # chunk 1 was DMA'd in the prologue
                xn, dman, _ = self.emit_dma_x(J + 1)
                xs[J + 1] = xn
                fills_h0 = fills_h0 + [dman]
            if J > 0:
                # chunk 1 is the tightest (16 fills over 16 iters): push
                # half of the previous outproj into the head1 window
                hi = 4 if J == 1 else 8
                fills_h0 = fills_h0 + self.ph3_units(J - 1, 1, 0, hi)
            # head1 fills: next chunk's q units + this chunk's h0 outproj
            fills_h1 = []
            if J + 1 < NCHUNK:
                fills_h1 += self.proj_q_units(J + 1, xs[J + 1])
            if J == 1:
                fills_h1 += self.ph3_units(0, 1, 4, 8)
            fills_h1 += self.ph3_units(J, 0, 0, 8)

            self._fill = self._fill + fills_h0
            self._rate = 4 if J == 0 else max(
                1, (len(self._fill) + nkb - 1) // nkb)
            st_carry = self.attention(J, 0, st0=st_carry, prefetch=(J, 1))

            self._fill = self._fill + fills_h1
            self._rate = max(1, (len(self._fill) + nkb - 1) // nkb)
            pf = (J + 1, 0) if J + 1 < NCHUNK else None
            st_carry = self.attention(J, 1, st0=st_carry, prefetch=pf,
                                      drain_early=(J == NCHUNK - 1))

        self.flush_fill()
        nc.sync.dma_start(self.dens[0:1, :], self.den0[:])
        nc.sync.dma_start(self.dens[1:2, :], self.den1[:])


def _emit(nc, tc, ctx, io):
    _Emitter(nc, tc, ctx, io).run()


_CACHED = None


def _build():
    global _CACHED
    if _CACHED is not None:
        return _CACHED
    nc = bacc.Bacc("TRN2", target_bir_lowering=False, debug=False,
                   enable_asserts=False, num_devices=NCORES)
    names = [
        ("xT", [D, S], BF16), ("wq_p", [128, 512], BF16),
        ("wk_p", [128, 512], BF16), ("wv_p", [128, 512], BF16),
        ("wo01", [128, 512], BF16),
        ("bqkv", [128, 3], F32), ("kbias", [128, 32], F32),
        ("trimask", [128, 128], BF16), ("ident2", [128, 64], BF16),
    ]
    aps = [nc.dram_tensor(n, sh, dt_, kind="ExternalInput").ap()
           for n, sh, dt_ in names]
    po0 = nc.dram_tensor("po0", [S, D], BF16, kind="ExternalOutput").ap()
    po1 = nc.dram_tensor("po1", [S, D], BF16, kind="ExternalOutput").ap()
    dens = nc.dram_tensor("dens", [2, S], F32, kind="ExternalOutput").ap()
    with tile.TileContext(nc) as tc, ExitStack() as ctx:
        _emit(nc, tc, ctx, aps + [po0, po1, dens])
    nc.compile()
    _CACHED = nc
    return nc


def _host_inputs(x, attention_mask, Wq, bq, Wk, bk, Wv, bv, Wo, bo):
    f = np.float32
    x = np.asarray(x, f)
    mask = np.asarray(attention_mask)
    Wq, Wk, Wv, Wo = (np.asarray(w, f) for w in (Wq, Wk, Wv, Wo))
    bq, bk, bv = (np.asarray(b_, f) for b_ in (bq, bk, bv))
    tri = np.triu(np.ones((128, 128), NPBF16))      # [k,q]: 1 where q >= k
    id2 = np.tile(np.eye(64, dtype=NPBF16), (2, 1))
    in_maps = []
    for c in range(NCORES):
        b = c // 4
        h0 = 2 * (c % 4)
        hsl = slice(64 * h0, 64 * h0 + 128)

        def pack_w(W):
            wt = W[hsl, :].T                        # [512, 128] = Wh^T
            return np.ascontiguousarray(
                wt.reshape(4, 128, 128).transpose(1, 0, 2)
                .reshape(128, 512).astype(NPBF16))

        wo_t = Wo[:, hsl].T.astype(NPBF16)           # [128, 512]
        kb = np.where(mask[b] != 0, f(0.0), f(NEG)).astype(f)
        in_maps.append({
            "xT": np.ascontiguousarray(x[b].T.astype(NPBF16)),
            "wq_p": pack_w(Wq), "wk_p": pack_w(Wk), "wv_p": pack_w(Wv),
            "wo01": np.ascontiguousarray(wo_t),
            "bqkv": np.ascontiguousarray(
                np.stack([bq[hsl], bk[hsl], bv[hsl]], axis=1)),
            "kbias": np.ascontiguousarray(kb.reshape(32, 128).T),
            "trimask": tri, "ident2": id2,
        })
    return in_maps


def _assemble(results, bo):
    out = np.zeros((B, S, D), np.float32)
    for c in range(NCORES):
        r = results[c]
        dens = r["dens"]
        part = (r["po0"].astype(np.float32) / dens[0][:, None]
                + r["po1"].astype(np.float32) / dens[1][:, None])
        out[c // 4] += part
    out += np.asarray(bo, np.float32)
    return out


def kernel(**inputs) -> np.ndarray:
    nc = _build()
    in_maps = _host_inputs(**inputs)
    last_err = None
    for attempt in range(3):
        try:
            res = bass_utils.run_bass_kernel_spmd(
                nc, in_maps, core_ids=list(range(NCORES)))
            out = _assemble(res.results, inputs["bo"])
        except Exception as e:  # transient NRT/axon device errors
            last_err = e
            continue
        if np.isfinite(out).all():
            return out
        last_err = RuntimeError("non-finite output")
    raise last_err


def run_traced(inputs, **kwargs):
    """test.py helper: run with NTFF tracing, return (out, BassKernelResults)."""
    nc = _build()
    in_maps = _host_inputs(**inputs)
    res = bass_utils.run_bass_kernel_spmd(
        nc, in_maps, core_ids=list(range(NCORES)), trace=True, **kwargs)
    return _assemble(res.results, inputs["bo"]), res


# revision 43
# speedup vs baseline: 1.2055x; 1.0085x over previous
"""Causal self-attention (B=2, S=4096, D=512, H=8) on 8 Trainium2 cores.

Sharding: core c handles batch b = c//4 and heads {2*(c%4), 2*(c%4)+1}.

Fused single-pipeline design: per query-chunk J the kernel runs attention
for head0 then head1 (k-major transposed scores, exp on ACT with the
padding mask folded into the per-partition bias), while the PE slack under
the ACT-bound steady state absorbs interleaved "filler" work: q/k/v
projections for chunk J+1, V transposes, and the q-major output projection
whose units self-append as soon as their half of the PV accumulator is
final.  Scores PSUM is double-buffered and QK for kb+1 issues before PV
for kb so ACT never waits; the first QK of the next head is prefetched
before the current head's PV tail.  Outputs are per-head undivided
projections po_h [S, 512] bf16 plus softmax denominators; the host
divides, sums heads/cores, and adds bo.

PSUM map (8 banks): st 2 bufs x [128,1024]f32 (4) | pv [65,1024]f32 (2)
| aux 2 bufs x [128,512]f32 shared by proj pieces / V transposes / outproj.

Head x row-group layout: qT/kT keep head0 on partitions 0-63, head1 on
64-127; dupq/dupk hold the swapped copy so head h can issue even kb blocks
on PE row group 0 and odd kb blocks on row group 64 (concurrent tiles).
"""

import sys

sys.path.insert(0, "/opt/trn_rl_repo")

from contextlib import ExitStack

import ml_dtypes
import numpy as np

import concourse.bass as bass
import concourse.tile as tile
from concourse import bacc, bass_utils, mybir

B, S, D = 2, 4096, 512
H, HD = 8, 64
NCORES = 8
F32 = mybir.dt.float32
BF16 = mybir.dt.bfloat16
EXP = mybir.ActivationFunctionType.Exp
NPBF16 = ml_dtypes.bfloat16

CHUNK = 1024                  # query-chunk width
NCHUNK = S // CHUNK           # 4
KBLK = 128                    # key block (partition dim)
KB_PER_CHUNK = CHUNK // KBLK  # 8
NEG = -1.0e30


def _pieces(col0):
    """Split [col0, CHUNK) into <=512-wide pieces aligned to 512 boundaries."""
    out = []
    c = col0
    while c < CHUNK:
        nxt = min(CHUNK, (c // 512 + 1) * 512)
        out.append((c, nxt))
        c = nxt
    return out


class _Emitter:
    def __init__(self, nc, tc, ctx, io):
        self.nc = nc
        (self.xT, self.wq_p, self.wk_p, self.wv_p, self.wo01, self.bqkv,
         self.kbias, self.trimask, self.ident2, self.po0, self.po1,
         self.dens) = io

        const = ctx.enter_context(tc.tile_pool(name="const", bufs=1))
        self.sb = ctx.enter_context(tc.tile_pool(name="sb", bufs=1))
        self.etp = ctx.enter_context(tc.tile_pool(name="etp", bufs=8))
        self.xp = ctx.enter_context(tc.tile_pool(name="xp", bufs=2))
        self.ps_st = ctx.enter_context(
            tc.tile_pool(name="ps_st", bufs=2, space="PSUM"))
        self.ps_pv = ctx.enter_context(
            tc.tile_pool(name="ps_pv", bufs=1, space="PSUM"))
        self.ps_aux = ctx.enter_context(
            tc.tile_pool(name="ps_aux", bufs=2, space="PSUM"))

        # constants / weights
        self.wq_sb = const.tile([128, 512], BF16, tag="wq")
        self.wk_sb = const.tile([128, 512], BF16, tag="wk")
        self.wv_sb = const.tile([128, 512], BF16, tag="wv")
        self.wo_sb = const.tile([128, 512], BF16, tag="wo")
        self.bqkv_sb = const.tile([128, 3], F32, tag="bqkv")
        self.kbias_sb = const.tile([128, 32], F32, tag="kbias")
        self.tri_sb = const.tile([128, 128], BF16, tag="tri")
        self.id2_sb = const.tile([128, 64], BF16, tag="id2")
        onesf_sb = const.tile([128, 1], F32, tag="onesf")
        nc.vector.memset(onesf_sb[:], 1.0)

        # persistent intermediates
        self.qT = self.sb.tile([128, S], BF16, tag="qT")
        self.kT = self.sb.tile([128, S], BF16, tag="kT")
        self.dupq = self.sb.tile([128, S], BF16, tag="dupq")
        self.dupk = self.sb.tile([128, S], BF16, tag="dupk")
        self.v0 = self.sb.tile([128, 32 * 65], BF16, tag="v0")
        self.v1 = self.sb.tile([128, 32 * 65], BF16, tag="v1")
        self.oT01 = self.sb.tile([128, S], BF16, tag="oT01")
        self.den0 = self.sb.tile([1, S], F32, tag="den0")
        self.den1 = self.sb.tile([1, S], F32, tag="den1")
        for vdst in (self.v0, self.v1):
            ones_col = vdst.rearrange("p (k c) -> p k c", c=65)[:, :, 64:65]
            nc.vector.tensor_copy(
                ones_col, onesf_sb[:].to_broadcast((128, 32, 1)))

        self._fill = []
        self._calls_left = 1

    # ---------------- filler machinery -----------------------------------
    def fill(self):
        for _ in range(self._rate):
            if self._fill:
                self._fill.pop(0)()

    def flush_fill(self):
        while self._fill:
            self._fill.pop(0)()

    def emit_dma_x(self, J):
        """Queue DMA of x chunk J (two 512-col halves). Returns x tiles."""
        nc = self.nc
        x_sb = []
        for ks in range(4):
            xt = self.xp.tile([128, CHUNK], BF16, tag=f"x{ks}")
            x_sb.append(xt)

        def half_unit(half):
            lo = half * 512
            for ks in range(4):
                nc.sync.dma_start(
                    x_sb[ks][:, lo:lo + 512],
                    self.xT[ks * 128:(ks + 1) * 128,
                            J * CHUNK + lo:J * CHUNK + lo + 512])

        def unit():
            half_unit(0)
            half_unit(1)
        return x_sb, unit, half_unit

    def _mk_proj(self, x_sb, w_sb, bcol, dest, base, lo):
        nc = self.nc

        def unit():
            ps = self.ps_aux.tile([128, 512], F32, tag="aux", name="ps")
            for ks in range(4):
                nc.tensor.matmul(
                    ps[:],
                    w_sb[:, ks * 128:(ks + 1) * 128],
                    x_sb[ks][:, lo:lo + 512],
                    start=(ks == 0), stop=(ks == 3))
            nc.vector.tensor_scalar_add(
                dest[:, base + lo:base + lo + 512], ps[:],
                self.bqkv_sb[:, bcol:bcol + 1])
        return [unit]

    def proj_q_units(self, J, x_sb):
        """q projection + dupq swap for chunk J (needed at chunk start)."""
        nc = self.nc
        csl = slice(J * CHUNK, (J + 1) * CHUNK)
        units = []
        for lo in (0, 512):
            units += self._mk_proj(x_sb, self.wq_sb, 0, self.qT,
                                   J * CHUNK, lo)

        def dupq_unit():
            nc.vector.tensor_copy(self.dupq[64:128, csl], self.qT[0:64, csl])
            nc.vector.tensor_copy(self.dupq[0:64, csl], self.qT[64:128, csl])
        units.append(dupq_unit)
        return units

    def proj_kv_units(self, J, x_sb):
        """k/v projections + dupk + V transpose/repack for chunk J (needed
        only once attention reaches chunk J's diagonal blocks)."""
        nc = self.nc
        csl = slice(J * CHUNK, (J + 1) * CHUNK)
        vT = self.sb.tile([128, CHUNK], BF16, tag="vTc", bufs=2)
        units = []
        for lo in (0, 512):
            units += self._mk_proj(x_sb, self.wk_sb, 1, self.kT,
                                   J * CHUNK, lo)

        def dupk_unit():
            nc.vector.tensor_copy(self.dupk[64:128, csl], self.kT[0:64, csl])
            nc.vector.tensor_copy(self.dupk[0:64, csl], self.kT[64:128, csl])
        units.append(dupk_unit)

        for lo in (0, 512):
            units += self._mk_proj(x_sb, self.wv_sb, 2, vT, 0, lo)

        # V -> k-major 65-stride blocks (per head)
        for hh, vdst in ((0, self.v0), (1, self.v1)):
            def unit(hh=hh, vdst=vdst):
                tr = self.ps_aux.tile([128, 512], BF16, tag="aux")
                for i in range(8):
                    nc.tensor.transpose(
                        tr[:, i * 64:(i + 1) * 64],
                        vT[hh * 64:(hh + 1) * 64, i * KBLK:(i + 1) * KBLK],
                        self.id2_sb[hh * 64:(hh + 1) * 64, :])
                dst = vdst[:, J * 8 * 65:(J + 1) * 8 * 65]
                dst = dst.rearrange("p (k c) -> p k c", c=65)[:, :, 0:64]
                nc.vector.tensor_copy(
                    dst, tr.rearrange("p (k c) -> p k c", c=64))
            units.append(unit)
        return units

    def ph3_units(self, J, hh, qb0, qb1, scalar_cast=False):
        """Output projection units for (chunk J, head hh), qb in [qb0,qb1).

        scalar_cast routes the PSUM->SBUF cast to the ACT engine — only
        valid when no exp work remains (it would delay the exp chain)."""
        nc = self.nc
        units = []
        hsl = slice(hh * 64, (hh + 1) * 64)
        dram = self.po0 if hh == 0 else self.po1
        for qb in range(qb0, qb1):
            def unit(qb=qb):
                q0 = J * CHUNK + qb * 128
                po = self.ps_aux.tile([128, 512], F32, tag="aux")
                nc.tensor.matmul(
                    po[:], self.oT01[hsl, q0:q0 + 128],
                    self.wo_sb[hsl, :], start=True, stop=True)
                posb = self.sb.tile([128, 512], BF16, tag="posb", bufs=4)
                if scalar_cast:
                    nc.scalar.copy(posb[:], po[:])
                else:
                    nc.vector.tensor_copy(posb[:], po[:])
                nc.sync.dma_start(dram[q0:q0 + 128, :], posb[:])
            units.append(unit)
        return units

    # ---------------- attention ------------------------------------------
    def _emit_qk(self, J, hh, kb):
        nc = self.nc
        p = kb - KB_PER_CHUNK * J
        col0 = KBLK * p if p >= 0 else 0
        # even kb -> natural layout at the head's home group; odd kb ->
        # swapped dup layout at the other group (concurrent PE tiles)
        if kb % 2 == 0:
            qsrc, ksrc, g = self.qT, self.kT, hh * 64
        else:
            qsrc, ksrc, g = self.dupq, self.dupk, 64 - hh * 64
        gs = slice(g, g + 64)
        st = self.ps_st.tile([128, CHUNK], F32, tag="st")
        for (a, b) in _pieces(col0):
            nc.tensor.matmul(
                st[:, a:b],
                ksrc[gs, kb * KBLK:(kb + 1) * KBLK],
                qsrc[gs, J * CHUNK + a:J * CHUNK + b],
                start=True, stop=True)
        return st

    def attention(self, J, hh, st0=None, prefetch=None, drain_early=False):
        """Attention for (chunk J, head hh). Returns the prefetched st of
        `prefetch` = (J', hh') if given (emitted before our PV tail)."""
        nc = self.nc
        vsb = self.v0 if hh == 0 else self.v1
        den = self.den0 if hh == 0 else self.den1
        hsl = slice(hh * 64, (hh + 1) * 64)
        pv = self.ps_pv.tile([65, CHUNK], F32, tag="pv")
        nkb = KB_PER_CHUNK * (J + 1)
        c0 = J * CHUNK
        st_next = None

        def emit_pv(kb, et, pieces):
            for (a, b) in pieces:
                last_a = (kb == KB_PER_CHUNK * J + 3 and a < 512)
                last_b = (kb == nkb - 1)
                nc.tensor.matmul(
                    pv[:, a:b],
                    vsb[:, kb * 65:(kb + 1) * 65],
                    et[:, a:b],
                    start=(kb == 0),
                    stop=(last_a if a < 512 else last_b))
            if drain_early and kb == KB_PER_CHUNK * J + 3:
                # pv[:, 0:512] final: drain it so its outproj can overlap
                nc.vector.tensor_copy(
                    self.oT01[hsl, c0:c0 + 512], pv[0:64, 0:512])
                nc.vector.tensor_copy(den[:, c0:c0 + 512], pv[64:65, 0:512])
                self._fill += self.ph3_units(J, hh, 0, 4)

        # software pipeline: PV lags one block so its et/tri deps are
        # already satisfied when the in-order PE queue reaches it.
        lagged = None
        st = st0 if st0 is not None else self._emit_qk(J, hh, 0)
        for kb in range(nkb):
            p = kb - KB_PER_CHUNK * J
            col0 = KBLK * p if p >= 0 else 0
            pieces = _pieces(col0)
            et = self.etp.tile([128, CHUNK], BF16, tag="et")
            nc.scalar.activation(
                et[:, col0:], st[:, col0:], EXP,
                bias=self.kbias_sb[:, kb:kb + 1], scale=0.125)
            if p >= 0:
                nc.vector.tensor_mul(
                    et[:, col0:col0 + KBLK], et[:, col0:col0 + KBLK],
                    self.tri_sb[:])
            if kb + 1 < nkb:
                st = self._emit_qk(J, hh, kb + 1)
            elif prefetch is not None:
                st_next = self._emit_qk(prefetch[0], prefetch[1], 0)
            self.fill()
            if lagged is not None:
                emit_pv(*lagged)
            lagged = (kb, et, pieces)
        emit_pv(*lagged)
        if drain_early:
            # the exp chain is finished: use the idle ACT engine for the
            # tail drain so the DVE (trimask/casts) isn't the serializer
            nc.scalar.copy(self.oT01[hsl, c0 + 512:c0 + CHUNK],
                           pv[0:64, 512:])
            nc.scalar.copy(den[:, c0 + 512:c0 + CHUNK], pv[64:65, 512:])
            self._fill += self.ph3_units(J, hh, 4, 8, scalar_cast=True)
        else:
            nc.vector.tensor_copy(self.oT01[hsl, c0:c0 + CHUNK], pv[0:64, :])
            nc.vector.tensor_copy(den[:, c0:c0 + CHUNK], pv[64:65, :])
        return st_next

    def run(self):
        nc = self.nc
        # prologue: x chunk-0 half0 + q/k weights land first so the first
        # projection starts as early as possible; remaining consts follow.
        x0, _, x0_half = self.emit_dma_x(0)
        x0_half(0)
        for t, a in ((self.wq_sb, self.wq_p), (self.bqkv_sb, self.bqkv),
                     (self.wk_sb, self.wk_p)):
            nc.sync.dma_start(t[:], a[:])
        x0_half(1)
        for t, a in ((self.wv_sb, self.wv_p), (self.id2_sb, self.ident2),
                     (self.kbias_sb, self.kbias), (self.tri_sb, self.trimask),
                     (self.wo_sb, self.wo01)):
            nc.sync.dma_start(t[:], a[:])
        # preload the exp table with a dummy 1-column activation so the
        # first real exp doesn't pay the ACT_TABLE_LOAD
        scratch = self.sb.tile([128, 1], F32, tag="scratch")
        nc.scalar.activation(scratch[:], self.bqkv_sb[:, 0:1], EXP,
                             bias=0.0, scale=0.0)
        # only what attention(0, h0) needs up front (q/k/dups); chunk 0's
        # v path runs as rate-4 fillers at the first iteration.  The first
        # QK issues as soon as q (full) and k half0 (covers key block 0)
        # exist; k half1 and the dup copies follow.
        q0u = self.proj_q_units(0, x0)
        kv0 = self.proj_kv_units(0, x0)
        for u in (q0u[0], kv0[0], q0u[1], kv0[1]):
            u()  # q half0, k half0, q half1, k half1
        st_carry = self._emit_qk(0, 0, 0)
        q0u[2]()  # dupq
        kv0[2]()  # dupk
        x1, dma1, _ = self.emit_dma_x(1)
        dma1()

        xs = {1: x1}
        for J in range(NCHUNK):
            nkb = KB_PER_CHUNK * (J + 1)
            # head0 fills: this chunk's k/v tail work (J=0: v path only),
            # next x DMA, and the previous chunk's head1 output projection.
            fills_h0 = kv0[3:] if J == 0 else self.proj_kv_units(J, xs[J])
            if 2 <= J + 1 < NCHUNK:  # chunk 1 was DMA'd in the prologue
                xn, dman, _ = self.emit_dma_x(J + 1)
                xs[J + 1] = xn
                fills_h0 = fills_h0 + [dman]
            if J > 0:
                # chunk 1 is the tightest (16 fills over 16 iters): push
                # half of the previous outproj into the head1 window
                hi = 4 if J == 1 else 8
                fills_h0 = fills_h0 + self.ph3_units(J - 1, 1, 0, hi)
            # head1 fills: next chunk's q units + this chunk's h0 outproj
            fills_h1 = []
            if J + 1 < NCHUNK:
                fills_h1 += self.proj_q_units(J + 1, xs[J + 1])
            if J == 1:
                fills_h1 += self.ph3_units(0, 1, 4, 8)
            fills_h1 += self.ph3_units(J, 0, 0, 8)

            self._fill = self._fill + fills_h0
            self._rate = 4 if J == 0 else max(
                1, (len(self._fill) + nkb - 1) // nkb)
            st_carry = self.attention(J, 0, st0=st_carry, prefetch=(J, 1))

            self._fill = self._fill + fills_h1
            self._rate = max(1, (len(self._fill) + nkb - 1) // nkb)
            pf = (J + 1, 0) if J + 1 < NCHUNK else None
            st_carry = self.attention(J, 1, st0=st_carry, prefetch=pf,
                                      drain_early=(J == NCHUNK - 1))

        self.flush_fill()
        nc.sync.dma_start(self.dens[0:1, :], self.den0[:])
        nc.sync.dma_start(self.dens[1:2, :], self.den1[:])


def _emit(nc, tc, ctx, io):
    _Emitter(nc, tc, ctx, io).run()


_CACHED = None


def _build():
    global _CACHED
    if _CACHED is not None:
        return _CACHED
    nc = bacc.Bacc("TRN2", target_bir_lowering=False, debug=False,
                   enable_asserts=False, num_devices=NCORES)
    names = [
        ("xT", [D, S], BF16), ("wq_p", [128, 512], BF16),
        ("wk_p", [128, 512], BF16), ("wv_p", [128, 512], BF16),
        ("wo01", [128, 512], BF16),
        ("bqkv", [128, 3], F32), ("kbias", [128, 32], F32),
        ("trimask", [128, 128], BF16), ("ident2", [128, 64], BF16),
    ]
    aps = [nc.dram_tensor(n, sh, dt_, kind="ExternalInput").ap()
           for n, sh, dt_ in names]
    po0 = nc.dram_tensor("po0", [S, D], BF16, kind="ExternalOutput").ap()
    po1 = nc.dram_tensor("po1", [S, D], BF16, kind="ExternalOutput").ap()
    dens = nc.dram_tensor("dens", [2, S], F32, kind="ExternalOutput").ap()
    with tile.TileContext(nc) as tc, ExitStack() as ctx:
        _emit(nc, tc, ctx, aps + [po0, po1, dens])
    nc.compile()
    _CACHED = nc
    return nc


def _host_inputs(x, attention_mask, Wq, bq, Wk, bk, Wv, bv, Wo, bo):
    f = np.float32
    x = np.asarray(x, f)
    mask = np.asarray(attention_mask)
    Wq, Wk, Wv, Wo = (np.asarray(w, f) for w in (Wq, Wk, Wv, Wo))
    bq, bk, bv = (np.asarray(b_, f) for b_ in (bq, bk, bv))
    tri = np.triu(np.ones((128, 128), NPBF16))      # [k,q]: 1 where q >= k
    id2 = np.tile(np.eye(64, dtype=NPBF16), (2, 1))
    in_maps = []
    for c in range(NCORES):
        b = c // 4
        h0 = 2 * (c % 4)
        hsl = slice(64 * h0, 64 * h0 + 128)

        def pack_w(W):
            wt = W[hsl, :].T                        # [512, 128] = Wh^T
            return np.ascontiguousarray(
                wt.reshape(4, 128, 128).transpose(1, 0, 2)
                .reshape(128, 512).astype(NPBF16))

        wo_t = Wo[:, hsl].T.astype(NPBF16)           # [128, 512]
        kb = np.where(mask[b] != 0, f(0.0), f(NEG)).astype(f)
        in_maps.append({
            "xT": np.ascontiguousarray(x[b].T.astype(NPBF16)),
            "wq_p": pack_w(Wq), "wk_p": pack_w(Wk), "wv_p": pack_w(Wv),
            "wo01": np.ascontiguousarray(wo_t),
            "bqkv": np.ascontiguousarray(
                np.stack([bq[hsl], bk[hsl], bv[hsl]], axis=1)),
            "kbias": np.ascontiguousarray(kb.reshape(32, 128).T),
            "trimask": tri, "ident2": id2,
        })
    return in_maps


def _assemble(results, bo):
    out = np.zeros((B, S, D), np.float32)
    for c in range(NCORES):
        r = results[c]
        dens = r["dens"]
        part = (r["po0"].astype(np.float32) / dens[0][:, None]
                + r["po1"].astype(np.float32) / dens[1][:, None])
        out[c // 4] += part
    out += np.asarray(bo, np.float32)
    return out


def kernel(**inputs) -> np.ndarray:
    nc = _build()
    in_maps = _host_inputs(**inputs)
    last_err = None
    for attempt in range(3):
        try:
            res = bass_utils.run_bass_kernel_spmd(
                nc, in_maps, core_ids=list(range(NCORES)))
            out = _assemble(res.results, inputs["bo"])
        except Exception as e:  # transient NRT/axon device errors
            last_err = e
            continue
        if np.isfinite(out).all():
            return out
        last_err = RuntimeError("non-finite output")
    raise last_err


def run_traced(inputs, **kwargs):
    """test.py helper: run with NTFF tracing, return (out, BassKernelResults)."""
    nc = _build()
    in_maps = _host_inputs(**inputs)
    res = bass_utils.run_bass_kernel_spmd(
        nc, in_maps, core_ids=list(range(NCORES)), trace=True, **kwargs)
    return _assemble(res.results, inputs["bo"]), res


# revision 44
# speedup vs baseline: 1.2080x; 1.0021x over previous
"""Causal self-attention (B=2, S=4096, D=512, H=8) on 8 Trainium2 cores.

Sharding: core c handles batch b = c//4 and heads {2*(c%4), 2*(c%4)+1}.

Fused single-pipeline design: per query-chunk J the kernel runs attention
for head0 then head1 (k-major transposed scores, exp on ACT with the
padding mask folded into the per-partition bias), while the PE slack under
the ACT-bound steady state absorbs interleaved "filler" work: q/k/v
projections for chunk J+1, V transposes, and the q-major output projection
whose units self-append as soon as their half of the PV accumulator is
final.  Scores PSUM is double-buffered and QK for kb+1 issues before PV
for kb so ACT never waits; the first QK of the next head is prefetched
before the current head's PV tail.  Outputs are per-head undivided
projections po_h [S, 512] bf16 plus softmax denominators; the host
divides, sums heads/cores, and adds bo.

PSUM map (8 banks): st 2 bufs x [128,1024]f32 (4) | pv [65,1024]f32 (2)
| aux 2 bufs x [128,512]f32 shared by proj pieces / V transposes / outproj.

Head x row-group layout: qT/kT keep head0 on partitions 0-63, head1 on
64-127; dupq/dupk hold the swapped copy so head h can issue even kb blocks
on PE row group 0 and odd kb blocks on row group 64 (concurrent tiles).
"""

import sys

sys.path.insert(0, "/opt/trn_rl_repo")

from contextlib import ExitStack

import ml_dtypes
import numpy as np

import concourse.bass as bass
import concourse.tile as tile
from concourse import bacc, bass_utils, mybir

B, S, D = 2, 4096, 512
H, HD = 8, 64
NCORES = 8
F32 = mybir.dt.float32
BF16 = mybir.dt.bfloat16
EXP = mybir.ActivationFunctionType.Exp
NPBF16 = ml_dtypes.bfloat16

CHUNK = 1024                  # query-chunk width
NCHUNK = S // CHUNK           # 4
KBLK = 128                    # key block (partition dim)
KB_PER_CHUNK = CHUNK // KBLK  # 8
NEG = -1.0e30


def _pieces(col0):
    """Split [col0, CHUNK) into <=512-wide pieces aligned to 512 boundaries."""
    out = []
    c = col0
    while c < CHUNK:
        nxt = min(CHUNK, (c // 512 + 1) * 512)
        out.append((c, nxt))
        c = nxt
    return out


class _Emitter:
    def __init__(self, nc, tc, ctx, io):
        self.nc = nc
        (self.xT, self.wq_p, self.wk_p, self.wv_p, self.wo01, self.bqkv,
         self.kbias, self.trimask, self.ident2, self.po0, self.po1,
         self.dens) = io

        const = ctx.enter_context(tc.tile_pool(name="const", bufs=1))
        self.sb = ctx.enter_context(tc.tile_pool(name="sb", bufs=1))
        self.etp = ctx.enter_context(tc.tile_pool(name="etp", bufs=8))
        self.xp = ctx.enter_context(tc.tile_pool(name="xp", bufs=2))
        self.ps_st = ctx.enter_context(
            tc.tile_pool(name="ps_st", bufs=2, space="PSUM"))
        self.ps_pv = ctx.enter_context(
            tc.tile_pool(name="ps_pv", bufs=1, space="PSUM"))
        self.ps_aux = ctx.enter_context(
            tc.tile_pool(name="ps_aux", bufs=2, space="PSUM"))

        # constants / weights
        self.wq_sb = const.tile([128, 512], BF16, tag="wq")
        self.wk_sb = const.tile([128, 512], BF16, tag="wk")
        self.wv_sb = const.tile([128, 512], BF16, tag="wv")
        self.wo_sb = const.tile([128, 512], BF16, tag="wo")
        self.bqkv_sb = const.tile([128, 3], F32, tag="bqkv")
        self.kbias_sb = const.tile([128, 32], F32, tag="kbias")
        self.tri_sb = const.tile([128, 128], BF16, tag="tri")
        self.id2_sb = const.tile([128, 64], BF16, tag="id2")
        onesf_sb = const.tile([128, 1], F32, tag="onesf")
        nc.vector.memset(onesf_sb[:], 1.0)

        # persistent intermediates
        self.qT = self.sb.tile([128, S], BF16, tag="qT")
        self.kT = self.sb.tile([128, S], BF16, tag="kT")
        self.dupq = self.sb.tile([128, S], BF16, tag="dupq")
        self.dupk = self.sb.tile([128, S], BF16, tag="dupk")
        self.v0 = self.sb.tile([128, 32 * 65], BF16, tag="v0")
        self.v1 = self.sb.tile([128, 32 * 65], BF16, tag="v1")
        self.oT01 = self.sb.tile([128, S], BF16, tag="oT01")
        self.den0 = self.sb.tile([1, S], F32, tag="den0")
        self.den1 = self.sb.tile([1, S], F32, tag="den1")
        for vdst in (self.v0, self.v1):
            ones_col = vdst.rearrange("p (k c) -> p k c", c=65)[:, :, 64:65]
            nc.vector.tensor_copy(
                ones_col, onesf_sb[:].to_broadcast((128, 32, 1)))

        self._fill = []
        self._calls_left = 1

    # ---------------- filler machinery -----------------------------------
    def fill(self):
        for _ in range(self._rate):
            if self._fill:
                self._fill.pop(0)()

    def flush_fill(self):
        while self._fill:
            self._fill.pop(0)()

    def emit_dma_x(self, J):
        """Queue DMA of x chunk J (two 512-col halves). Returns x tiles."""
        nc = self.nc
        x_sb = []
        for ks in range(4):
            xt = self.xp.tile([128, CHUNK], BF16, tag=f"x{ks}")
            x_sb.append(xt)

        def half_unit(half):
            lo = half * 512
            for ks in range(4):
                nc.sync.dma_start(
                    x_sb[ks][:, lo:lo + 512],
                    self.xT[ks * 128:(ks + 1) * 128,
                            J * CHUNK + lo:J * CHUNK + lo + 512])

        def unit():
            half_unit(0)
            half_unit(1)
        return x_sb, unit, half_unit

    def _mk_proj(self, x_sb, w_sb, bcol, dest, base, lo):
        nc = self.nc

        def unit():
            ps = self.ps_aux.tile([128, 512], F32, tag="aux", name="ps")
            for ks in range(4):
                nc.tensor.matmul(
                    ps[:],
                    w_sb[:, ks * 128:(ks + 1) * 128],
                    x_sb[ks][:, lo:lo + 512],
                    start=(ks == 0), stop=(ks == 3))
            nc.vector.tensor_scalar_add(
                dest[:, base + lo:base + lo + 512], ps[:],
                self.bqkv_sb[:, bcol:bcol + 1])
        return [unit]

    def proj_q_units(self, J, x_sb):
        """q projection + dupq swap for chunk J (needed at chunk start)."""
        nc = self.nc
        csl = slice(J * CHUNK, (J + 1) * CHUNK)
        units = []
        for lo in (0, 512):
            units += self._mk_proj(x_sb, self.wq_sb, 0, self.qT,
                                   J * CHUNK, lo)

        def dupq_unit():
            nc.vector.tensor_copy(self.dupq[64:128, csl], self.qT[0:64, csl])
            nc.vector.tensor_copy(self.dupq[0:64, csl], self.qT[64:128, csl])
        units.append(dupq_unit)
        return units

    def proj_kv_units(self, J, x_sb):
        """k/v projections + dupk + V transpose/repack for chunk J (needed
        only once attention reaches chunk J's diagonal blocks)."""
        nc = self.nc
        csl = slice(J * CHUNK, (J + 1) * CHUNK)
        vT = self.sb.tile([128, CHUNK], BF16, tag="vTc", bufs=2)
        units = []
        for lo in (0, 512):
            units += self._mk_proj(x_sb, self.wk_sb, 1, self.kT,
                                   J * CHUNK, lo)

        def dupk_unit():
            nc.vector.tensor_copy(self.dupk[64:128, csl], self.kT[0:64, csl])
            nc.vector.tensor_copy(self.dupk[0:64, csl], self.kT[64:128, csl])
        units.append(dupk_unit)

        for lo in (0, 512):
            units += self._mk_proj(x_sb, self.wv_sb, 2, vT, 0, lo)

        # V -> k-major 65-stride blocks (per head)
        for hh, vdst in ((0, self.v0), (1, self.v1)):
            def unit(hh=hh, vdst=vdst):
                tr = self.ps_aux.tile([128, 512], BF16, tag="aux")
                for i in range(8):
                    nc.tensor.transpose(
                        tr[:, i * 64:(i + 1) * 64],
                        vT[hh * 64:(hh + 1) * 64, i * KBLK:(i + 1) * KBLK],
                        self.id2_sb[hh * 64:(hh + 1) * 64, :])
                dst = vdst[:, J * 8 * 65:(J + 1) * 8 * 65]
                dst = dst.rearrange("p (k c) -> p k c", c=65)[:, :, 0:64]
                nc.vector.tensor_copy(
                    dst, tr.rearrange("p (k c) -> p k c", c=64))
            units.append(unit)
        return units

    def ph3_units(self, J, hh, qb0, qb1, scalar_cast=False):
        """Output projection units for (chunk J, head hh), qb in [qb0,qb1).

        scalar_cast routes the PSUM->SBUF cast to the ACT engine — only
        valid when no exp work remains (it would delay the exp chain)."""
        nc = self.nc
        units = []
        hsl = slice(hh * 64, (hh + 1) * 64)
        dram = self.po0 if hh == 0 else self.po1
        for qb in range(qb0, qb1):
            def unit(qb=qb):
                q0 = J * CHUNK + qb * 128
                po = self.ps_aux.tile([128, 512], F32, tag="aux")
                nc.tensor.matmul(
                    po[:], self.oT01[hsl, q0:q0 + 128],
                    self.wo_sb[hsl, :], start=True, stop=True)
                posb = self.sb.tile([128, 512], BF16, tag="posb", bufs=4)
                if scalar_cast:
                    nc.scalar.copy(posb[:], po[:])
                else:
                    nc.vector.tensor_copy(posb[:], po[:])
                nc.sync.dma_start(dram[q0:q0 + 128, :], posb[:])
            units.append(unit)
        return units

    # ---------------- attention ------------------------------------------
    def _emit_qk(self, J, hh, kb):
        nc = self.nc
        p = kb - KB_PER_CHUNK * J
        col0 = KBLK * p if p >= 0 else 0
        # even kb -> natural layout at the head's home group; odd kb ->
        # swapped dup layout at the other group (concurrent PE tiles)
        if kb % 2 == 0:
            qsrc, ksrc, g = self.qT, self.kT, hh * 64
        else:
            qsrc, ksrc, g = self.dupq, self.dupk, 64 - hh * 64
        gs = slice(g, g + 64)
        st = self.ps_st.tile([128, CHUNK], F32, tag="st")
        for (a, b) in _pieces(col0):
            nc.tensor.matmul(
                st[:, a:b],
                ksrc[gs, kb * KBLK:(kb + 1) * KBLK],
                qsrc[gs, J * CHUNK + a:J * CHUNK + b],
                start=True, stop=True)
        return st

    def attention(self, J, hh, st0=None, prefetch=None, drain_early=False):
        """Attention for (chunk J, head hh). Returns the prefetched st of
        `prefetch` = (J', hh') if given (emitted before our PV tail)."""
        nc = self.nc
        vsb = self.v0 if hh == 0 else self.v1
        den = self.den0 if hh == 0 else self.den1
        hsl = slice(hh * 64, (hh + 1) * 64)
        pv = self.ps_pv.tile([65, CHUNK], F32, tag="pv")
        nkb = KB_PER_CHUNK * (J + 1)
        c0 = J * CHUNK
        st_next = None

        def emit_pv(kb, et, pieces):
            for (a, b) in pieces:
                last_a = (kb == KB_PER_CHUNK * J + 3 and a < 512)
                last_b = (kb == nkb - 1)
                nc.tensor.matmul(
                    pv[:, a:b],
                    vsb[:, kb * 65:(kb + 1) * 65],
                    et[:, a:b],
                    start=(kb == 0),
                    stop=(last_a if a < 512 else last_b))
            if drain_early and kb == KB_PER_CHUNK * J + 3:
                # pv[:, 0:512] final: drain it so its outproj can overlap
                nc.vector.tensor_copy(
                    self.oT01[hsl, c0:c0 + 512], pv[0:64, 0:512])
                nc.vector.tensor_copy(den[:, c0:c0 + 512], pv[64:65, 0:512])
                self._fill += self.ph3_units(J, hh, 0, 4)

        # software pipeline: PV lags one block so its et/tri deps are
        # already satisfied when the in-order PE queue reaches it.
        lagged = None
        st = st0 if st0 is not None else self._emit_qk(J, hh, 0)
        for kb in range(nkb):
            p = kb - KB_PER_CHUNK * J
            col0 = KBLK * p if p >= 0 else 0
            pieces = _pieces(col0)
            et = self.etp.tile([128, CHUNK], BF16, tag="et")
            nc.scalar.activation(
                et[:, col0:], st[:, col0:], EXP,
                bias=self.kbias_sb[:, kb:kb + 1], scale=0.125)
            if p >= 0:
                nc.vector.tensor_mul(
                    et[:, col0:col0 + KBLK], et[:, col0:col0 + KBLK],
                    self.tri_sb[:])
            if kb + 1 < nkb:
                st = self._emit_qk(J, hh, kb + 1)
            elif prefetch is not None:
                st_next = self._emit_qk(prefetch[0], prefetch[1], 0)
            self.fill()
            if lagged is not None:
                emit_pv(*lagged)
            lagged = (kb, et, pieces)
        emit_pv(*lagged)
        if drain_early:
            # the exp chain is finished: use the idle ACT engine for the
            # tail drain so the DVE (trimask/casts) isn't the serializer
            nc.scalar.copy(self.oT01[hsl, c0 + 512:c0 + CHUNK],
                           pv[0:64, 512:])
            nc.scalar.copy(den[:, c0 + 512:c0 + CHUNK], pv[64:65, 512:])
            self._fill += self.ph3_units(J, hh, 4, 8, scalar_cast=True)
        else:
            nc.vector.tensor_copy(self.oT01[hsl, c0:c0 + CHUNK], pv[0:64, :])
            nc.vector.tensor_copy(den[:, c0:c0 + CHUNK], pv[64:65, :])
        return st_next

    def run(self):
        nc = self.nc
        # prologue: x chunk-0 half0 + q/k weights land first so the first
        # projection starts as early as possible; remaining consts follow.
        x0, _, x0_half = self.emit_dma_x(0)
        x0_half(0)
        for t, a in ((self.wq_sb, self.wq_p), (self.bqkv_sb, self.bqkv),
                     (self.wk_sb, self.wk_p)):
            nc.sync.dma_start(t[:], a[:])
        x0_half(1)
        for t, a in ((self.wv_sb, self.wv_p), (self.id2_sb, self.ident2),
                     (self.kbias_sb, self.kbias), (self.tri_sb, self.trimask),
                     (self.wo_sb, self.wo01)):
            nc.sync.dma_start(t[:], a[:])
        # preload the exp table with a dummy 1-column activation so the
        # first real exp doesn't pay the ACT_TABLE_LOAD
        scratch = self.sb.tile([128, 1], F32, tag="scratch")
        nc.scalar.activation(scratch[:], self.bqkv_sb[:, 0:1], EXP,
                             bias=0.0, scale=0.0)
        # only what attention(0, h0) needs up front (q/k/dups); chunk 0's
        # v path runs as rate-4 fillers at the first iteration.  The first
        # QK issues as soon as q (full) and k half0 (covers key block 0)
        # exist; k half1 and the dup copies follow.
        q0u = self.proj_q_units(0, x0)
        kv0 = self.proj_kv_units(0, x0)
        for u in (q0u[0], kv0[0], q0u[1], kv0[1]):
            u()  # q half0, k half0, q half1, k half1
        st_carry = self._emit_qk(0, 0, 0)
        q0u[2]()  # dupq
        kv0[2]()  # dupk
        x1, dma1, _ = self.emit_dma_x(1)
        dma1()

        # Output-projection batches have no downstream consumer on device,
        # so defer each into a later window with filler slack: chunks 0-1
        # run a PE deficit (many fills, few iterations) while chunks 2-3
        # have 8-24 spare iterations.  (chunk, head) -> batches to emit.
        ph3_sched = {
            (1, 1): [(0, 0)],
            (2, 0): [(0, 1)],
            (2, 1): [(1, 0)],
            (3, 0): [(1, 1), (2, 0)],
            (3, 1): [(2, 1), (3, 0)],
        }
        xs = {1: x1}
        for J in range(NCHUNK):
            nkb = KB_PER_CHUNK * (J + 1)
            # head0 fills: this chunk's k/v tail work (J=0: v path only),
            # the next x DMA, and any scheduled outproj batches.
            fills_h0 = kv0[3:] if J == 0 else self.proj_kv_units(J, xs[J])
            if 2 <= J + 1 < NCHUNK:  # chunk 1 was DMA'd in the prologue
                xn, dman, _ = self.emit_dma_x(J + 1)
                xs[J + 1] = xn
                fills_h0 = fills_h0 + [dman]
            for bj, bh in ph3_sched.get((J, 0), []):
                fills_h0 = fills_h0 + self.ph3_units(bj, bh, 0, 8)
            # head1 fills: next chunk's q units + scheduled outproj batches
            fills_h1 = []
            if J + 1 < NCHUNK:
                fills_h1 += self.proj_q_units(J + 1, xs[J + 1])
            for bj, bh in ph3_sched.get((J, 1), []):
                fills_h1 += self.ph3_units(bj, bh, 0, 8)

            self._fill = self._fill + fills_h0
            self._rate = 4 if J == 0 else max(
                1, (len(self._fill) + nkb - 1) // nkb)
            st_carry = self.attention(J, 0, st0=st_carry, prefetch=(J, 1))

            self._fill = self._fill + fills_h1
            self._rate = max(1, (len(self._fill) + nkb - 1) // nkb)
            pf = (J + 1, 0) if J + 1 < NCHUNK else None
            st_carry = self.attention(J, 1, st0=st_carry, prefetch=pf,
                                      drain_early=(J == NCHUNK - 1))

        self.flush_fill()
        nc.sync.dma_start(self.dens[0:1, :], self.den0[:])
        nc.sync.dma_start(self.dens[1:2, :], self.den1[:])


def _emit(nc, tc, ctx, io):
    _Emitter(nc, tc, ctx, io).run()


_CACHED = None


def _build():
    global _CACHED
    if _CACHED is not None:
        return _CACHED
    nc = bacc.Bacc("TRN2", target_bir_lowering=False, debug=False,
                   enable_asserts=False, num_devices=NCORES)
    names = [
        ("xT", [D, S], BF16), ("wq_p", [128, 512], BF16),
        ("wk_p", [128, 512], BF16), ("wv_p", [128, 512], BF16),
        ("wo01", [128, 512], BF16),
        ("bqkv", [128, 3], F32), ("kbias", [128, 32], F32),
        ("trimask", [128, 128], BF16), ("ident2", [128, 64], BF16),
    ]
    aps = [nc.dram_tensor(n, sh, dt_, kind="ExternalInput").ap()
           for n, sh, dt_ in names]
    po0 = nc.dram_tensor("po0", [S, D], BF16, kind="ExternalOutput").ap()
    po1 = nc.dram_tensor("po1", [S, D], BF16, kind="ExternalOutput").ap()
    dens = nc.dram_tensor("dens", [2, S], F32, kind="ExternalOutput").ap()
    with tile.TileContext(nc) as tc, ExitStack() as ctx:
        _emit(nc, tc, ctx, aps + [po0, po1, dens])
    nc.compile()
    _CACHED = nc
    return nc


def _host_inputs(x, attention_mask, Wq, bq, Wk, bk, Wv, bv, Wo, bo):
    f = np.float32
    x = np.asarray(x, f)
    mask = np.asarray(attention_mask)
    Wq, Wk, Wv, Wo = (np.asarray(w, f) for w in (Wq, Wk, Wv, Wo))
    bq, bk, bv = (np.asarray(b_, f) for b_ in (bq, bk, bv))
    tri = np.triu(np.ones((128, 128), NPBF16))      # [k,q]: 1 where q >= k
    id2 = np.tile(np.eye(64, dtype=NPBF16), (2, 1))
    in_maps = []
    for c in range(NCORES):
        b = c // 4
        h0 = 2 * (c % 4)
        hsl = slice(64 * h0, 64 * h0 + 128)

        def pack_w(W):
            wt = W[hsl, :].T                        # [512, 128] = Wh^T
            return np.ascontiguousarray(
                wt.reshape(4, 128, 128).transpose(1, 0, 2)
                .reshape(128, 512).astype(NPBF16))

        wo_t = Wo[:, hsl].T.astype(NPBF16)           # [128, 512]
        kb = np.where(mask[b] != 0, f(0.0), f(NEG)).astype(f)
        in_maps.append({
            "xT": np.ascontiguousarray(x[b].T.astype(NPBF16)),
            "wq_p": pack_w(Wq), "wk_p": pack_w(Wk), "wv_p": pack_w(Wv),
            "wo01": np.ascontiguousarray(wo_t),
            "bqkv": np.ascontiguousarray(
                np.stack([bq[hsl], bk[hsl], bv[hsl]], axis=1)),
            "kbias": np.ascontiguousarray(kb.reshape(32, 128).T),
            "trimask": tri, "ident2": id2,
        })
    return in_maps


def _assemble(results, bo):
    out = np.zeros((B, S, D), np.float32)
    for c in range(NCORES):
        r = results[c]
        dens = r["dens"]
        part = (r["po0"].astype(np.float32) / dens[0][:, None]
                + r["po1"].astype(np.float32) / dens[1][:, None])
        out[c // 4] += part
    out += np.asarray(bo, np.float32)
    return out


def kernel(**inputs) -> np.ndarray:
    nc = _build()
    in_maps = _host_inputs(**inputs)
    last_err = None
    for attempt in range(3):
        try:
            res = bass_utils.run_bass_kernel_spmd(
                nc, in_maps, core_ids=list(range(NCORES)))
            out = _assemble(res.results, inputs["bo"])
        except Exception as e:  # transient NRT/axon device errors
            last_err = e
            continue
        if np.isfinite(out).all():
            return out
        last_err = RuntimeError("non-finite output")
    raise last_err


def run_traced(inputs, **kwargs):
    """test.py helper: run with NTFF tracing, return (out, BassKernelResults)."""
    nc = _build()
    in_maps = _host_inputs(**inputs)
    res = bass_utils.run_bass_kernel_spmd(
        nc, in_maps, core_ids=list(range(NCORES)), trace=True, **kwargs)
    return _assemble(res.results, inputs["bo"]), res


# revision 45
# speedup vs baseline: 1.2102x; 1.0018x over previous
"""Causal self-attention (B=2, S=4096, D=512, H=8) on 8 Trainium2 cores.

Sharding: core c handles batch b = c//4 and heads {2*(c%4), 2*(c%4)+1}.

Fused single-pipeline design: per query-chunk J the kernel runs attention
for head0 then head1 (k-major transposed scores, exp on ACT with the
padding mask folded into the per-partition bias), while the PE slack under
the ACT-bound steady state absorbs interleaved "filler" work: q/k/v
projections for chunk J+1, V transposes, and the q-major output projection
whose units self-append as soon as their half of the PV accumulator is
final.  Scores PSUM is double-buffered and QK for kb+1 issues before PV
for kb so ACT never waits; the first QK of the next head is prefetched
before the current head's PV tail.  Outputs are per-head undivided
projections po_h [S, 512] bf16 plus softmax denominators; the host
divides, sums heads/cores, and adds bo.

PSUM map (8 banks): st 2 bufs x [128,1024]f32 (4) | pv [65,1024]f32 (2)
| aux 2 bufs x [128,512]f32 shared by proj pieces / V transposes / outproj.

Head x row-group layout: qT/kT keep head0 on partitions 0-63, head1 on
64-127; dupq/dupk hold the swapped copy so head h can issue even kb blocks
on PE row group 0 and odd kb blocks on row group 64 (concurrent tiles).
"""

import sys

sys.path.insert(0, "/opt/trn_rl_repo")

from contextlib import ExitStack

import ml_dtypes
import numpy as np

import concourse.bass as bass
import concourse.tile as tile
from concourse import bacc, bass_utils, mybir

B, S, D = 2, 4096, 512
H, HD = 8, 64
NCORES = 8
F32 = mybir.dt.float32
BF16 = mybir.dt.bfloat16
EXP = mybir.ActivationFunctionType.Exp
NPBF16 = ml_dtypes.bfloat16

CHUNK = 1024                  # query-chunk width
NCHUNK = S // CHUNK           # 4
KBLK = 128                    # key block (partition dim)
KB_PER_CHUNK = CHUNK // KBLK  # 8
NEG = -1.0e30


def _pieces(col0):
    """Split [col0, CHUNK) into <=512-wide pieces aligned to 512 boundaries."""
    out = []
    c = col0
    while c < CHUNK:
        nxt = min(CHUNK, (c // 512 + 1) * 512)
        out.append((c, nxt))
        c = nxt
    return out


class _Emitter:
    def __init__(self, nc, tc, ctx, io):
        self.nc = nc
        (self.xT, self.wq_p, self.wk_p, self.wv_p, self.wo01, self.bqkv,
         self.kbias, self.trimask, self.ident2, self.po0, self.po1,
         self.dens) = io

        const = ctx.enter_context(tc.tile_pool(name="const", bufs=1))
        self.sb = ctx.enter_context(tc.tile_pool(name="sb", bufs=1))
        self.etp = ctx.enter_context(tc.tile_pool(name="etp", bufs=8))
        self.xp = ctx.enter_context(tc.tile_pool(name="xp", bufs=2))
        self.ps_st = ctx.enter_context(
            tc.tile_pool(name="ps_st", bufs=2, space="PSUM"))
        self.ps_pv = ctx.enter_context(
            tc.tile_pool(name="ps_pv", bufs=1, space="PSUM"))
        self.ps_aux = ctx.enter_context(
            tc.tile_pool(name="ps_aux", bufs=2, space="PSUM"))

        # constants / weights
        self.wq_sb = const.tile([128, 512], BF16, tag="wq")
        self.wk_sb = const.tile([128, 512], BF16, tag="wk")
        self.wv_sb = const.tile([128, 512], BF16, tag="wv")
        self.wo_sb = const.tile([128, 512], BF16, tag="wo")
        self.bqkv_sb = const.tile([128, 3], F32, tag="bqkv")
        self.kbias_sb = const.tile([128, 32], F32, tag="kbias")
        self.tri_sb = const.tile([128, 128], BF16, tag="tri")
        self.id2_sb = const.tile([128, 64], BF16, tag="id2")
        onesf_sb = const.tile([128, 1], F32, tag="onesf")
        nc.vector.memset(onesf_sb[:], 1.0)

        # persistent intermediates
        self.qT = self.sb.tile([128, S], BF16, tag="qT")
        self.kT = self.sb.tile([128, S], BF16, tag="kT")
        self.dupq = self.sb.tile([128, S], BF16, tag="dupq")
        self.dupk = self.sb.tile([128, S], BF16, tag="dupk")
        self.v0 = self.sb.tile([128, 32 * 65], BF16, tag="v0")
        self.v1 = self.sb.tile([128, 32 * 65], BF16, tag="v1")
        self.oT01 = self.sb.tile([128, S], BF16, tag="oT01")
        self.den0 = self.sb.tile([1, S], F32, tag="den0")
        self.den1 = self.sb.tile([1, S], F32, tag="den1")
        for vdst in (self.v0, self.v1):
            ones_col = vdst.rearrange("p (k c) -> p k c", c=65)[:, :, 64:65]
            nc.vector.tensor_copy(
                ones_col, onesf_sb[:].to_broadcast((128, 32, 1)))

        self._fill = []
        self._calls_left = 1

    # ---------------- filler machinery -----------------------------------
    def fill(self):
        for _ in range(self._rate):
            if self._fill:
                self._fill.pop(0)()

    def flush_fill(self):
        while self._fill:
            self._fill.pop(0)()

    def emit_dma_x(self, J):
        """Queue DMA of x chunk J (two 512-col halves). Returns x tiles."""
        nc = self.nc
        x_sb = []
        for ks in range(4):
            xt = self.xp.tile([128, CHUNK], BF16, tag=f"x{ks}")
            x_sb.append(xt)

        def half_unit(half):
            lo = half * 512
            for ks in range(4):
                nc.sync.dma_start(
                    x_sb[ks][:, lo:lo + 512],
                    self.xT[ks * 128:(ks + 1) * 128,
                            J * CHUNK + lo:J * CHUNK + lo + 512])

        def unit():
            half_unit(0)
            half_unit(1)
        return x_sb, unit, half_unit

    def _mk_proj(self, x_sb, w_sb, bcol, dest, base, lo):
        nc = self.nc

        def unit():
            ps = self.ps_aux.tile([128, 512], F32, tag="aux", name="ps")
            for ks in range(4):
                nc.tensor.matmul(
                    ps[:],
                    w_sb[:, ks * 128:(ks + 1) * 128],
                    x_sb[ks][:, lo:lo + 512],
                    start=(ks == 0), stop=(ks == 3))
            nc.vector.tensor_scalar_add(
                dest[:, base + lo:base + lo + 512], ps[:],
                self.bqkv_sb[:, bcol:bcol + 1])
        return [unit]

    def proj_q_units(self, J, x_sb):
        """q projection + dupq swap for chunk J (needed at chunk start)."""
        nc = self.nc
        csl = slice(J * CHUNK, (J + 1) * CHUNK)
        units = []
        for lo in (0, 512):
            units += self._mk_proj(x_sb, self.wq_sb, 0, self.qT,
                                   J * CHUNK, lo)

        def dupq_unit():
            nc.vector.tensor_copy(self.dupq[64:128, csl], self.qT[0:64, csl])
            nc.vector.tensor_copy(self.dupq[0:64, csl], self.qT[64:128, csl])
        units.append(dupq_unit)
        return units

    def proj_kv_units(self, J, x_sb):
        """k/v projections + dupk + V transpose/repack for chunk J (needed
        only once attention reaches chunk J's diagonal blocks)."""
        nc = self.nc
        csl = slice(J * CHUNK, (J + 1) * CHUNK)
        vT = self.sb.tile([128, CHUNK], BF16, tag="vTc", bufs=2)
        units = []
        for lo in (0, 512):
            units += self._mk_proj(x_sb, self.wk_sb, 1, self.kT,
                                   J * CHUNK, lo)

        def dupk_unit():
            nc.vector.tensor_copy(self.dupk[64:128, csl], self.kT[0:64, csl])
            nc.vector.tensor_copy(self.dupk[0:64, csl], self.kT[64:128, csl])
        units.append(dupk_unit)

        for lo in (0, 512):
            units += self._mk_proj(x_sb, self.wv_sb, 2, vT, 0, lo)

        # V -> k-major 65-stride blocks (per head)
        for hh, vdst in ((0, self.v0), (1, self.v1)):
            def unit(hh=hh, vdst=vdst):
                tr = self.ps_aux.tile([128, 512], BF16, tag="aux")
                for i in range(8):
                    nc.tensor.transpose(
                        tr[:, i * 64:(i + 1) * 64],
                        vT[hh * 64:(hh + 1) * 64, i * KBLK:(i + 1) * KBLK],
                        self.id2_sb[hh * 64:(hh + 1) * 64, :])
                dst = vdst[:, J * 8 * 65:(J + 1) * 8 * 65]
                dst = dst.rearrange("p (k c) -> p k c", c=65)[:, :, 0:64]
                nc.vector.tensor_copy(
                    dst, tr.rearrange("p (k c) -> p k c", c=64))
            units.append(unit)
        return units

    def ph3_units(self, J, hh, qb0, qb1, scalar_cast=False):
        """Output projection units for (chunk J, head hh), qb in [qb0,qb1).

        scalar_cast routes the PSUM->SBUF cast to the ACT engine — only
        valid when no exp work remains (it would delay the exp chain)."""
        nc = self.nc
        units = []
        hsl = slice(hh * 64, (hh + 1) * 64)
        dram = self.po0 if hh == 0 else self.po1
        for qb in range(qb0, qb1):
            def unit(qb=qb):
                q0 = J * CHUNK + qb * 128
                po = self.ps_aux.tile([128, 512], F32, tag="aux")
                nc.tensor.matmul(
                    po[:], self.oT01[hsl, q0:q0 + 128],
                    self.wo_sb[hsl, :], start=True, stop=True)
                posb = self.sb.tile([128, 512], BF16, tag="posb", bufs=4)
                if scalar_cast:
                    nc.scalar.copy(posb[:], po[:])
                else:
                    nc.vector.tensor_copy(posb[:], po[:])
                nc.sync.dma_start(dram[q0:q0 + 128, :], posb[:])
            units.append(unit)
        return units

    # ---------------- attention ------------------------------------------
    def _emit_qk(self, J, hh, kb):
        nc = self.nc
        p = kb - KB_PER_CHUNK * J
        col0 = KBLK * p if p >= 0 else 0
        # even kb -> natural layout at the head's home group; odd kb ->
        # swapped dup layout at the other group (concurrent PE tiles)
        if kb % 2 == 0:
            qsrc, ksrc, g = self.qT, self.kT, hh * 64
        else:
            qsrc, ksrc, g = self.dupq, self.dupk, 64 - hh * 64
        gs = slice(g, g + 64)
        st = self.ps_st.tile([128, CHUNK], F32, tag="st")
        for (a, b) in _pieces(col0):
            nc.tensor.matmul(
                st[:, a:b],
                ksrc[gs, kb * KBLK:(kb + 1) * KBLK],
                qsrc[gs, J * CHUNK + a:J * CHUNK + b],
                start=True, stop=True)
        return st

    def attention(self, J, hh, st0=None, prefetch=None, drain_early=False):
        """Attention for (chunk J, head hh). Returns the prefetched st of
        `prefetch` = (J', hh') if given (emitted before our PV tail)."""
        nc = self.nc
        vsb = self.v0 if hh == 0 else self.v1
        den = self.den0 if hh == 0 else self.den1
        hsl = slice(hh * 64, (hh + 1) * 64)
        pv = self.ps_pv.tile([65, CHUNK], F32, tag="pv")
        nkb = KB_PER_CHUNK * (J + 1)
        c0 = J * CHUNK
        st_next = None

        def emit_pv(kb, et, pieces):
            for (a, b) in pieces:
                last_a = (kb == KB_PER_CHUNK * J + 3 and a < 512)
                last_b = (kb == nkb - 1)
                nc.tensor.matmul(
                    pv[:, a:b],
                    vsb[:, kb * 65:(kb + 1) * 65],
                    et[:, a:b],
                    start=(kb == 0),
                    stop=(last_a if a < 512 else last_b))
            if drain_early and kb == KB_PER_CHUNK * J + 3:
                # pv[:, 0:512] final: drain it so its outproj can overlap
                nc.vector.tensor_copy(
                    self.oT01[hsl, c0:c0 + 512], pv[0:64, 0:512])
                nc.vector.tensor_copy(den[:, c0:c0 + 512], pv[64:65, 0:512])
                self._fill += self.ph3_units(J, hh, 0, 4)

        # software pipeline: PV lags one block so its et/tri deps are
        # already satisfied when the in-order PE queue reaches it.
        lagged = None
        st = st0 if st0 is not None else self._emit_qk(J, hh, 0)
        for kb in range(nkb):
            p = kb - KB_PER_CHUNK * J
            col0 = KBLK * p if p >= 0 else 0
            pieces = _pieces(col0)
            et = self.etp.tile([128, CHUNK], BF16, tag="et")
            nc.scalar.activation(
                et[:, col0:], st[:, col0:], EXP,
                bias=self.kbias_sb[:, kb:kb + 1], scale=0.125)
            if p >= 0:
                nc.vector.tensor_mul(
                    et[:, col0:col0 + KBLK], et[:, col0:col0 + KBLK],
                    self.tri_sb[:])
            if kb + 1 < nkb:
                st = self._emit_qk(J, hh, kb + 1)
            elif prefetch is not None:
                st_next = self._emit_qk(prefetch[0], prefetch[1], 0)
            self.fill()
            if lagged is not None:
                emit_pv(*lagged)
            lagged = (kb, et, pieces)
        emit_pv(*lagged)
        if drain_early:
            # the exp chain is finished: use the idle ACT engine for the
            # tail drain so the DVE (trimask/casts) isn't the serializer
            nc.scalar.copy(self.oT01[hsl, c0 + 512:c0 + CHUNK],
                           pv[0:64, 512:])
            nc.scalar.copy(den[:, c0 + 512:c0 + CHUNK], pv[64:65, 512:])
            self._fill += self.ph3_units(J, hh, 4, 8, scalar_cast=True)
        else:
            nc.vector.tensor_copy(self.oT01[hsl, c0:c0 + CHUNK], pv[0:64, :])
            nc.vector.tensor_copy(den[:, c0:c0 + CHUNK], pv[64:65, :])
        return st_next

    def run(self):
        nc = self.nc
        # prologue: x chunk-0 half0 + q/k weights land first so the first
        # projection starts as early as possible; remaining consts follow.
        x0, _, x0_half = self.emit_dma_x(0)
        x0_half(0)
        for t, a in ((self.wq_sb, self.wq_p), (self.bqkv_sb, self.bqkv),
                     (self.wk_sb, self.wk_p)):
            nc.sync.dma_start(t[:], a[:])
        x0_half(1)
        for t, a in ((self.wv_sb, self.wv_p), (self.id2_sb, self.ident2),
                     (self.kbias_sb, self.kbias), (self.tri_sb, self.trimask),
                     (self.wo_sb, self.wo01)):
            nc.sync.dma_start(t[:], a[:])
        # preload the exp table with a dummy 1-column activation so the
        # first real exp doesn't pay the ACT_TABLE_LOAD
        scratch = self.sb.tile([128, 1], F32, tag="scratch")
        nc.scalar.activation(scratch[:], self.bqkv_sb[:, 0:1], EXP,
                             bias=0.0, scale=0.0)
        # warm the PE p-state during the input-DMA wait: cold matmuls run
        # ~634ns/512col vs ~380ns warm, and the ramp needs sustained use
        warm = self.sb.tile([128, 512], BF16, tag="warm")
        nc.vector.memset(warm[:], 0.0)
        for _ in range(10):
            wps = self.ps_aux.tile([128, 512], F32, tag="aux", name="wps")
            nc.tensor.matmul(wps[:], warm[:, 0:128], warm[:],
                             start=True, stop=True)
        # only what attention(0, h0) needs up front (q/k/dups); chunk 0's
        # v path runs as rate-4 fillers at the first iteration.  The first
        # QK issues as soon as q (full) and k half0 (covers key block 0)
        # exist; k half1 and the dup copies follow.
        q0u = self.proj_q_units(0, x0)
        kv0 = self.proj_kv_units(0, x0)
        for u in (q0u[0], kv0[0], q0u[1], kv0[1]):
            u()  # q half0, k half0, q half1, k half1
        st_carry = self._emit_qk(0, 0, 0)
        q0u[2]()  # dupq
        kv0[2]()  # dupk
        x1, dma1, _ = self.emit_dma_x(1)
        dma1()

        # Output-projection batches have no downstream consumer on device,
        # so defer each into a later window with filler slack: chunks 0-1
        # run a PE deficit (many fills, few iterations) while chunks 2-3
        # have 8-24 spare iterations.  (chunk, head) -> batches to emit.
        ph3_sched = {
            (1, 1): [(0, 0)],
            (2, 0): [(0, 1)],
            (2, 1): [(1, 0)],
            (3, 0): [(1, 1), (2, 0)],
            (3, 1): [(2, 1), (3, 0)],
        }
        xs = {1: x1}
        for J in range(NCHUNK):
            nkb = KB_PER_CHUNK * (J + 1)
            # head0 fills: this chunk's k/v tail work (J=0: v path only),
            # the next x DMA, and any scheduled outproj batches.
            fills_h0 = kv0[3:] if J == 0 else self.proj_kv_units(J, xs[J])
            if 2 <= J + 1 < NCHUNK:  # chunk 1 was DMA'd in the prologue
                xn, dman, _ = self.emit_dma_x(J + 1)
                xs[J + 1] = xn
                fills_h0 = fills_h0 + [dman]
            for bj, bh in ph3_sched.get((J, 0), []):
                fills_h0 = fills_h0 + self.ph3_units(bj, bh, 0, 8)
            # head1 fills: next chunk's q units + scheduled outproj batches
            fills_h1 = []
            if J + 1 < NCHUNK:
                fills_h1 += self.proj_q_units(J + 1, xs[J + 1])
            for bj, bh in ph3_sched.get((J, 1), []):
                fills_h1 += self.ph3_units(bj, bh, 0, 8)

            self._fill = self._fill + fills_h0
            self._rate = 4 if J == 0 else max(
                1, (len(self._fill) + nkb - 1) // nkb)
            st_carry = self.attention(J, 0, st0=st_carry, prefetch=(J, 1))

            self._fill = self._fill + fills_h1
            self._rate = max(1, (len(self._fill) + nkb - 1) // nkb)
            pf = (J + 1, 0) if J + 1 < NCHUNK else None
            st_carry = self.attention(J, 1, st0=st_carry, prefetch=pf,
                                      drain_early=(J == NCHUNK - 1))

        self.flush_fill()
        nc.sync.dma_start(self.dens[0:1, :], self.den0[:])
        nc.sync.dma_start(self.dens[1:2, :], self.den1[:])


def _emit(nc, tc, ctx, io):
    _Emitter(nc, tc, ctx, io).run()


_CACHED = None


def _build():
    global _CACHED
    if _CACHED is not None:
        return _CACHED
    nc = bacc.Bacc("TRN2", target_bir_lowering=False, debug=False,
                   enable_asserts=False, num_devices=NCORES)
    names = [
        ("xT", [D, S], BF16), ("wq_p", [128, 512], BF16),
        ("wk_p", [128, 512], BF16), ("wv_p", [128, 512], BF16),
        ("wo01", [128, 512], BF16),
        ("bqkv", [128, 3], F32), ("kbias", [128, 32], F32),
        ("trimask", [128, 128], BF16), ("ident2", [128, 64], BF16),
    ]
    aps = [nc.dram_tensor(n, sh, dt_, kind="ExternalInput").ap()
           for n, sh, dt_ in names]
    po0 = nc.dram_tensor("po0", [S, D], BF16, kind="ExternalOutput").ap()
    po1 = nc.dram_tensor("po1", [S, D], BF16, kind="ExternalOutput").ap()
    dens = nc.dram_tensor("dens", [2, S], F32, kind="ExternalOutput").ap()
    with tile.TileContext(nc) as tc, ExitStack() as ctx:
        _emit(nc, tc, ctx, aps + [po0, po1, dens])
    nc.compile()
    _CACHED = nc
    return nc


def _host_inputs(x, attention_mask, Wq, bq, Wk, bk, Wv, bv, Wo, bo):
    f = np.float32
    x = np.asarray(x, f)
    mask = np.asarray(attention_mask)
    Wq, Wk, Wv, Wo = (np.asarray(w, f) for w in (Wq, Wk, Wv, Wo))
    bq, bk, bv = (np.asarray(b_, f) for b_ in (bq, bk, bv))
    tri = np.triu(np.ones((128, 128), NPBF16))      # [k,q]: 1 where q >= k
    id2 = np.tile(np.eye(64, dtype=NPBF16), (2, 1))
    in_maps = []
    for c in range(NCORES):
        b = c // 4
        h0 = 2 * (c % 4)
        hsl = slice(64 * h0, 64 * h0 + 128)

        def pack_w(W):
            wt = W[hsl, :].T                        # [512, 128] = Wh^T
            return np.ascontiguousarray(
                wt.reshape(4, 128, 128).transpose(1, 0, 2)
                .reshape(128, 512).astype(NPBF16))

        wo_t = Wo[:, hsl].T.astype(NPBF16)           # [128, 512]
        kb = np.where(mask[b] != 0, f(0.0), f(NEG)).astype(f)
        in_maps.append({
            "xT": np.ascontiguousarray(x[b].T.astype(NPBF16)),
            "wq_p": pack_w(Wq), "wk_p": pack_w(Wk), "wv_p": pack_w(Wv),
            "wo01": np.ascontiguousarray(wo_t),
            "bqkv": np.ascontiguousarray(
                np.stack([bq[hsl], bk[hsl], bv[hsl]], axis=1)),
            "kbias": np.ascontiguousarray(kb.reshape(32, 128).T),
            "trimask": tri, "ident2": id2,
        })
    return in_maps


def _assemble(results, bo):
    out = np.zeros((B, S, D), np.float32)
    for c in range(NCORES):
        r = results[c]
        dens = r["dens"]
        part = (r["po0"].astype(np.float32) / dens[0][:, None]
                + r["po1"].astype(np.float32) / dens[1][:, None])
        out[c // 4] += part
    out += np.asarray(bo, np.float32)
    return out


def kernel(**inputs) -> np.ndarray:
    nc = _build()
    in_maps = _host_inputs(**inputs)
    last_err = None
    for attempt in range(3):
        try:
            res = bass_utils.run_bass_kernel_spmd(
                nc, in_maps, core_ids=list(range(NCORES)))
            out = _assemble(res.results, inputs["bo"])
        except Exception as e:  # transient NRT/axon device errors
            last_err = e
            continue
        if np.isfinite(out).all():
            return out
        last_err = RuntimeError("non-finite output")
    raise last_err


def run_traced(inputs, **kwargs):
    """test.py helper: run with NTFF tracing, return (out, BassKernelResults)."""
    nc = _build()
    in_maps = _host_inputs(**inputs)
    res = bass_utils.run_bass_kernel_spmd(
        nc, in_maps, core_ids=list(range(NCORES)), trace=True, **kwargs)
    return _assemble(res.results, inputs["bo"]), res


# revision 46
# speedup vs baseline: 1.2204x; 1.0084x over previous
"""Causal self-attention (B=2, S=4096, D=512, H=8) on 8 Trainium2 cores.

Sharding: core c handles batch b = c//4 and heads {2*(c%4), 2*(c%4)+1}.

Fused single-pipeline design: per query-chunk J the kernel runs attention
for head0 then head1 (k-major transposed scores, exp on ACT with the
padding mask folded into the per-partition bias), while the PE slack under
the ACT-bound steady state absorbs interleaved "filler" work: q/k/v
projections for chunk J+1, V transposes, and the q-major output projection
whose units self-append as soon as their half of the PV accumulator is
final.  Scores PSUM is double-buffered and QK for kb+1 issues before PV
for kb so ACT never waits; the first QK of the next head is prefetched
before the current head's PV tail.  Outputs are per-head undivided
projections po_h [S, 512] bf16 plus softmax denominators; the host
divides, sums heads/cores, and adds bo.

PSUM map (8 banks): st 2 bufs x [128,1024]f32 (4) | pv [65,1024]f32 (2)
| aux 2 bufs x [128,512]f32 shared by proj pieces / V transposes / outproj.

Head x row-group layout: qT/kT keep head0 on partitions 0-63, head1 on
64-127; dupq/dupk hold the swapped copy so head h can issue even kb blocks
on PE row group 0 and odd kb blocks on row group 64 (concurrent tiles).
"""

import sys

sys.path.insert(0, "/opt/trn_rl_repo")

from contextlib import ExitStack

import ml_dtypes
import numpy as np

import concourse.bass as bass
import concourse.tile as tile
from concourse import bacc, bass_utils, mybir

B, S, D = 2, 4096, 512
H, HD = 8, 64
NCORES = 8
F32 = mybir.dt.float32
BF16 = mybir.dt.bfloat16
EXP = mybir.ActivationFunctionType.Exp
NPBF16 = ml_dtypes.bfloat16

CHUNK = 1024                  # query-chunk width
NCHUNK = S // CHUNK           # 4
KBLK = 128                    # key block (partition dim)
KB_PER_CHUNK = CHUNK // KBLK  # 8
NEG = -1.0e30


def _pieces(col0):
    """Split [col0, CHUNK) into <=512-wide pieces aligned to 512 boundaries."""
    out = []
    c = col0
    while c < CHUNK:
        nxt = min(CHUNK, (c // 512 + 1) * 512)
        out.append((c, nxt))
        c = nxt
    return out


class _Emitter:
    def __init__(self, nc, tc, ctx, io):
        self.nc = nc
        (self.xT, self.wq_p, self.wk_p, self.wv_p, self.wo01, self.bqkv,
         self.kbias, self.trimask, self.ident2, self.po0, self.po1,
         self.dens) = io

        const = ctx.enter_context(tc.tile_pool(name="const", bufs=1))
        self.sb = ctx.enter_context(tc.tile_pool(name="sb", bufs=1))
        self.etp = ctx.enter_context(tc.tile_pool(name="etp", bufs=8))
        self.xp = ctx.enter_context(tc.tile_pool(name="xp", bufs=2))
        self.ps_st = ctx.enter_context(
            tc.tile_pool(name="ps_st", bufs=2, space="PSUM"))
        self.ps_pv = ctx.enter_context(
            tc.tile_pool(name="ps_pv", bufs=1, space="PSUM"))
        self.ps_aux = ctx.enter_context(
            tc.tile_pool(name="ps_aux", bufs=2, space="PSUM"))

        # constants / weights
        self.wq_sb = const.tile([128, 512], BF16, tag="wq")
        self.wk_sb = const.tile([128, 512], BF16, tag="wk")
        self.wv_sb = const.tile([128, 512], BF16, tag="wv")
        self.wo_sb = const.tile([128, 512], BF16, tag="wo")
        self.bqkv_sb = const.tile([128, 3], F32, tag="bqkv")
        self.kbias_sb = const.tile([128, 32], F32, tag="kbias")
        self.tri_sb = const.tile([128, 128], BF16, tag="tri")
        self.id2_sb = const.tile([128, 64], BF16, tag="id2")
        onesf_sb = const.tile([128, 1], F32, tag="onesf")
        nc.vector.memset(onesf_sb[:], 1.0)

        # persistent intermediates
        self.qT = self.sb.tile([128, S], BF16, tag="qT")
        self.kT = self.sb.tile([128, S], BF16, tag="kT")
        self.dupq = self.sb.tile([128, S], BF16, tag="dupq")
        self.dupk = self.sb.tile([128, S], BF16, tag="dupk")
        self.v0 = self.sb.tile([128, 32 * 65], BF16, tag="v0")
        self.v1 = self.sb.tile([128, 32 * 65], BF16, tag="v1")
        self.oT01 = self.sb.tile([128, S], BF16, tag="oT01")
        self.den0 = self.sb.tile([1, S], F32, tag="den0")
        self.den1 = self.sb.tile([1, S], F32, tag="den1")
        for vdst in (self.v0, self.v1):
            ones_col = vdst.rearrange("p (k c) -> p k c", c=65)[:, :, 64:65]
            nc.vector.tensor_copy(
                ones_col, onesf_sb[:].to_broadcast((128, 32, 1)))

        self._fill = []
        self._calls_left = 1

    # ---------------- filler machinery -----------------------------------
    def fill(self):
        for _ in range(self._rate):
            if self._fill:
                self._fill.pop(0)()

    def flush_fill(self):
        while self._fill:
            self._fill.pop(0)()

    def emit_dma_x(self, J):
        """Queue DMA of x chunk J (two 512-col halves). Returns x tiles."""
        nc = self.nc
        x_sb = []
        for ks in range(4):
            xt = self.xp.tile([128, CHUNK], BF16, tag=f"x{ks}")
            x_sb.append(xt)

        def half_unit(half):
            lo = half * 512
            for ks in range(4):
                nc.sync.dma_start(
                    x_sb[ks][:, lo:lo + 512],
                    self.xT[ks * 128:(ks + 1) * 128,
                            J * CHUNK + lo:J * CHUNK + lo + 512])

        def unit():
            half_unit(0)
            half_unit(1)
        return x_sb, unit, half_unit

    def _mk_proj(self, x_sb, w_sb, bcol, dest, base, lo):
        nc = self.nc

        def unit():
            ps = self.ps_aux.tile([128, 512], F32, tag="aux", name="ps")
            for ks in range(4):
                nc.tensor.matmul(
                    ps[:],
                    w_sb[:, ks * 128:(ks + 1) * 128],
                    x_sb[ks][:, lo:lo + 512],
                    start=(ks == 0), stop=(ks == 3))
            nc.vector.tensor_scalar_add(
                dest[:, base + lo:base + lo + 512], ps[:],
                self.bqkv_sb[:, bcol:bcol + 1])
        return [unit]

    def proj_q_units(self, J, x_sb):
        """q projection + dupq swap for chunk J (needed at chunk start)."""
        nc = self.nc
        csl = slice(J * CHUNK, (J + 1) * CHUNK)
        units = []
        for lo in (0, 512):
            units += self._mk_proj(x_sb, self.wq_sb, 0, self.qT,
                                   J * CHUNK, lo)

        def dupq_unit():
            nc.vector.tensor_copy(self.dupq[64:128, csl], self.qT[0:64, csl])
            nc.vector.tensor_copy(self.dupq[0:64, csl], self.qT[64:128, csl])
        units.append(dupq_unit)
        return units

    def proj_kv_units(self, J, x_sb):
        """k/v projections + dupk + V transpose/repack for chunk J (needed
        only once attention reaches chunk J's diagonal blocks)."""
        nc = self.nc
        csl = slice(J * CHUNK, (J + 1) * CHUNK)
        vT = self.sb.tile([128, CHUNK], BF16, tag="vTc", bufs=2)
        units = []
        for lo in (0, 512):
            units += self._mk_proj(x_sb, self.wk_sb, 1, self.kT,
                                   J * CHUNK, lo)

        def dupk_unit():
            nc.vector.tensor_copy(self.dupk[64:128, csl], self.kT[0:64, csl])
            nc.vector.tensor_copy(self.dupk[0:64, csl], self.kT[64:128, csl])
        units.append(dupk_unit)

        for lo in (0, 512):
            units += self._mk_proj(x_sb, self.wv_sb, 2, vT, 0, lo)

        # V -> k-major 65-stride blocks (per head)
        for hh, vdst in ((0, self.v0), (1, self.v1)):
            def unit(hh=hh, vdst=vdst):
                tr = self.ps_aux.tile([128, 512], BF16, tag="aux")
                for i in range(8):
                    nc.tensor.transpose(
                        tr[:, i * 64:(i + 1) * 64],
                        vT[hh * 64:(hh + 1) * 64, i * KBLK:(i + 1) * KBLK],
                        self.id2_sb[hh * 64:(hh + 1) * 64, :])
                dst = vdst[:, J * 8 * 65:(J + 1) * 8 * 65]
                dst = dst.rearrange("p (k c) -> p k c", c=65)[:, :, 0:64]
                nc.vector.tensor_copy(
                    dst, tr.rearrange("p (k c) -> p k c", c=64))
            units.append(unit)
        return units

    def ph3_units(self, J, hh, qb0, qb1, scalar_cast=False):
        """Output projection units for (chunk J, head hh), qb in [qb0,qb1).

        scalar_cast routes the PSUM->SBUF cast to the ACT engine — only
        valid when no exp work remains (it would delay the exp chain)."""
        nc = self.nc
        units = []
        hsl = slice(hh * 64, (hh + 1) * 64)
        dram = self.po0 if hh == 0 else self.po1
        for qb in range(qb0, qb1):
            def unit(qb=qb):
                q0 = J * CHUNK + qb * 128
                po = self.ps_aux.tile([128, 512], F32, tag="aux")
                nc.tensor.matmul(
                    po[:], self.oT01[hsl, q0:q0 + 128],
                    self.wo_sb[hsl, :], start=True, stop=True)
                posb = self.sb.tile([128, 512], BF16, tag="posb", bufs=4)
                if scalar_cast:
                    nc.scalar.copy(posb[:], po[:])
                else:
                    nc.vector.tensor_copy(posb[:], po[:])
                nc.sync.dma_start(dram[q0:q0 + 128, :], posb[:])
            units.append(unit)
        return units

    # ---------------- attention ------------------------------------------
    def _emit_qk(self, J, hh, kb):
        nc = self.nc
        p = kb - KB_PER_CHUNK * J
        col0 = KBLK * p if p >= 0 else 0
        # even kb -> natural layout at the head's home group; odd kb ->
        # swapped dup layout at the other group (concurrent PE tiles)
        if kb % 2 == 0:
            qsrc, ksrc, g = self.qT, self.kT, hh * 64
        else:
            qsrc, ksrc, g = self.dupq, self.dupk, 64 - hh * 64
        gs = slice(g, g + 64)
        st = self.ps_st.tile([128, CHUNK], F32, tag="st")
        for (a, b) in _pieces(col0):
            nc.tensor.matmul(
                st[:, a:b],
                ksrc[gs, kb * KBLK:(kb + 1) * KBLK],
                qsrc[gs, J * CHUNK + a:J * CHUNK + b],
                start=True, stop=True)
        return st

    def attention(self, J, hh, st0=None, prefetch=None, drain_early=False):
        """Attention for (chunk J, head hh). Returns the prefetched st of
        `prefetch` = (J', hh') if given (emitted before our PV tail)."""
        nc = self.nc
        vsb = self.v0 if hh == 0 else self.v1
        den = self.den0 if hh == 0 else self.den1
        hsl = slice(hh * 64, (hh + 1) * 64)
        pv = self.ps_pv.tile([65, CHUNK], F32, tag="pv")
        nkb = KB_PER_CHUNK * (J + 1)
        c0 = J * CHUNK
        st_next = None

        def emit_pv(kb, et, pieces):
            for (a, b) in pieces:
                last_a = (kb == KB_PER_CHUNK * J + 3 and a < 512)
                last_b = (kb == nkb - 1)
                nc.tensor.matmul(
                    pv[:, a:b],
                    vsb[:, kb * 65:(kb + 1) * 65],
                    et[:, a:b],
                    start=(kb == 0),
                    stop=(last_a if a < 512 else last_b))
            if drain_early and kb == KB_PER_CHUNK * J + 3:
                # pv[:, 0:512] final: drain it so its outproj can overlap
                nc.vector.tensor_copy(
                    self.oT01[hsl, c0:c0 + 512], pv[0:64, 0:512])
                nc.vector.tensor_copy(den[:, c0:c0 + 512], pv[64:65, 0:512])
                self._fill += self.ph3_units(J, hh, 0, 4)

        # software pipeline: PV lags one block so its et/tri deps are
        # already satisfied when the in-order PE queue reaches it.
        lagged = None
        st = st0 if st0 is not None else self._emit_qk(J, hh, 0)
        for kb in range(nkb):
            p = kb - KB_PER_CHUNK * J
            col0 = KBLK * p if p >= 0 else 0
            pieces = _pieces(col0)
            et = self.etp.tile([128, CHUNK], BF16, tag="et")
            nc.scalar.activation(
                et[:, col0:], st[:, col0:], EXP,
                bias=self.kbias_sb[:, kb:kb + 1], scale=0.125)
            if p >= 0:
                nc.vector.tensor_mul(
                    et[:, col0:col0 + KBLK], et[:, col0:col0 + KBLK],
                    self.tri_sb[:])
            if kb + 1 < nkb:
                st = self._emit_qk(J, hh, kb + 1)
                # prefetch the next head's first QK one iteration early so
                # it isn't queued behind this head's PV tail on the PE
                if prefetch is not None and kb + 2 == nkb:
                    st_next = self._emit_qk(prefetch[0], prefetch[1], 0)
            elif prefetch is not None and st_next is None:
                st_next = self._emit_qk(prefetch[0], prefetch[1], 0)
            self.fill()
            if lagged is not None:
                emit_pv(*lagged)
            lagged = (kb, et, pieces)
        emit_pv(*lagged)
        if drain_early:
            # the exp chain is finished: use the idle ACT engine for the
            # tail drain so the DVE (trimask/casts) isn't the serializer
            nc.scalar.copy(self.oT01[hsl, c0 + 512:c0 + CHUNK],
                           pv[0:64, 512:])
            nc.scalar.copy(den[:, c0 + 512:c0 + CHUNK], pv[64:65, 512:])
            self._fill += self.ph3_units(J, hh, 4, 8, scalar_cast=True)
        else:
            nc.vector.tensor_copy(self.oT01[hsl, c0:c0 + CHUNK], pv[0:64, :])
            nc.vector.tensor_copy(den[:, c0:c0 + CHUNK], pv[64:65, :])
        return st_next

    def run(self):
        nc = self.nc
        # prologue: x chunk-0 half0 + q/k weights land first so the first
        # projection starts as early as possible; remaining consts follow.
        x0, _, x0_half = self.emit_dma_x(0)
        x0_half(0)
        for t, a in ((self.wq_sb, self.wq_p), (self.bqkv_sb, self.bqkv),
                     (self.wk_sb, self.wk_p)):
            nc.sync.dma_start(t[:], a[:])
        x0_half(1)
        for t, a in ((self.wv_sb, self.wv_p), (self.id2_sb, self.ident2),
                     (self.kbias_sb, self.kbias), (self.tri_sb, self.trimask),
                     (self.wo_sb, self.wo01)):
            nc.sync.dma_start(t[:], a[:])
        # preload the exp table with a dummy 1-column activation so the
        # first real exp doesn't pay the ACT_TABLE_LOAD
        scratch = self.sb.tile([128, 1], F32, tag="scratch")
        nc.scalar.activation(scratch[:], self.bqkv_sb[:, 0:1], EXP,
                             bias=0.0, scale=0.0)
        # warm the PE p-state during the input-DMA wait: cold matmuls run
        # ~634ns/512col vs ~380ns warm, and the ramp needs sustained use
        warm = self.sb.tile([128, 512], BF16, tag="warm")
        nc.vector.memset(warm[:], 0.0)
        for _ in range(10):
            wps = self.ps_aux.tile([128, 512], F32, tag="aux", name="wps")
            nc.tensor.matmul(wps[:], warm[:, 0:128], warm[:],
                             start=True, stop=True)
        # only what attention(0, h0) needs up front (q/k/dups); chunk 0's
        # v path runs as rate-4 fillers at the first iteration.  The first
        # QK issues as soon as q (full) and k half0 (covers key block 0)
        # exist; k half1 and the dup copies follow.
        q0u = self.proj_q_units(0, x0)
        kv0 = self.proj_kv_units(0, x0)
        for u in (q0u[0], kv0[0], q0u[1], kv0[1]):
            u()  # q half0, k half0, q half1, k half1
        st_carry = self._emit_qk(0, 0, 0)
        q0u[2]()  # dupq
        kv0[2]()  # dupk
        x1, dma1, _ = self.emit_dma_x(1)
        dma1()

        # Output-projection batches have no downstream consumer on device,
        # so defer each into a later window with filler slack: chunks 0-1
        # run a PE deficit (many fills, few iterations) while chunks 2-3
        # have 8-24 spare iterations.  (chunk, head) -> batches to emit.
        ph3_sched = {
            (1, 1): [(0, 0)],
            (2, 0): [(0, 1)],
            (2, 1): [(1, 0)],
            (3, 0): [(1, 1), (2, 0)],
            (3, 1): [(2, 1), (3, 0)],
        }
        xs = {1: x1}
        for J in range(NCHUNK):
            nkb = KB_PER_CHUNK * (J + 1)
            # head0 fills: this chunk's k/v tail work (J=0: v path only),
            # the next x DMA, and any scheduled outproj batches.
            fills_h0 = kv0[3:] if J == 0 else self.proj_kv_units(J, xs[J])
            if 2 <= J + 1 < NCHUNK:  # chunk 1 was DMA'd in the prologue
                xn, dman, _ = self.emit_dma_x(J + 1)
                xs[J + 1] = xn
                fills_h0 = fills_h0 + [dman]
            for bj, bh in ph3_sched.get((J, 0), []):
                fills_h0 = fills_h0 + self.ph3_units(bj, bh, 0, 8)
            # head1 fills: next chunk's q units + scheduled outproj batches
            fills_h1 = []
            if J + 1 < NCHUNK:
                fills_h1 += self.proj_q_units(J + 1, xs[J + 1])
            for bj, bh in ph3_sched.get((J, 1), []):
                fills_h1 += self.ph3_units(bj, bh, 0, 8)

            self._fill = self._fill + fills_h0
            self._rate = 4 if J == 0 else max(
                1, (len(self._fill) + nkb - 1) // nkb)
            st_carry = self.attention(J, 0, st0=st_carry, prefetch=(J, 1))

            self._fill = self._fill + fills_h1
            self._rate = max(1, (len(self._fill) + nkb - 1) // nkb)
            pf = (J + 1, 0) if J + 1 < NCHUNK else None
            st_carry = self.attention(J, 1, st0=st_carry, prefetch=pf,
                                      drain_early=(J == NCHUNK - 1))

        self.flush_fill()
        nc.sync.dma_start(self.dens[0:1, :], self.den0[:])
        nc.sync.dma_start(self.dens[1:2, :], self.den1[:])


def _emit(nc, tc, ctx, io):
    _Emitter(nc, tc, ctx, io).run()


_CACHED = None


def _build():
    global _CACHED
    if _CACHED is not None:
        return _CACHED
    nc = bacc.Bacc("TRN2", target_bir_lowering=False, debug=False,
                   enable_asserts=False, num_devices=NCORES)
    names = [
        ("xT", [D, S], BF16), ("wq_p", [128, 512], BF16),
        ("wk_p", [128, 512], BF16), ("wv_p", [128, 512], BF16),
        ("wo01", [128, 512], BF16),
        ("bqkv", [128, 3], F32), ("kbias", [128, 32], F32),
        ("trimask", [128, 128], BF16), ("ident2", [128, 64], BF16),
    ]
    aps = [nc.dram_tensor(n, sh, dt_, kind="ExternalInput").ap()
           for n, sh, dt_ in names]
    po0 = nc.dram_tensor("po0", [S, D], BF16, kind="ExternalOutput").ap()
    po1 = nc.dram_tensor("po1", [S, D], BF16, kind="ExternalOutput").ap()
    dens = nc.dram_tensor("dens", [2, S], F32, kind="ExternalOutput").ap()
    with tile.TileContext(nc) as tc, ExitStack() as ctx:
        _emit(nc, tc, ctx, aps + [po0, po1, dens])
    nc.compile()
    _CACHED = nc
    return nc


def _host_inputs(x, attention_mask, Wq, bq, Wk, bk, Wv, bv, Wo, bo):
    f = np.float32
    x = np.asarray(x, f)
    mask = np.asarray(attention_mask)
    Wq, Wk, Wv, Wo = (np.asarray(w, f) for w in (Wq, Wk, Wv, Wo))
    bq, bk, bv = (np.asarray(b_, f) for b_ in (bq, bk, bv))
    tri = np.triu(np.ones((128, 128), NPBF16))      # [k,q]: 1 where q >= k
    id2 = np.tile(np.eye(64, dtype=NPBF16), (2, 1))
    in_maps = []
    for c in range(NCORES):
        b = c // 4
        h0 = 2 * (c % 4)
        hsl = slice(64 * h0, 64 * h0 + 128)

        def pack_w(W):
            wt = W[hsl, :].T                        # [512, 128] = Wh^T
            return np.ascontiguousarray(
                wt.reshape(4, 128, 128).transpose(1, 0, 2)
                .reshape(128, 512).astype(NPBF16))

        wo_t = Wo[:, hsl].T.astype(NPBF16)           # [128, 512]
        kb = np.where(mask[b] != 0, f(0.0), f(NEG)).astype(f)
        in_maps.append({
            "xT": np.ascontiguousarray(x[b].T.astype(NPBF16)),
            "wq_p": pack_w(Wq), "wk_p": pack_w(Wk), "wv_p": pack_w(Wv),
            "wo01": np.ascontiguousarray(wo_t),
            "bqkv": np.ascontiguousarray(
                np.stack([bq[hsl], bk[hsl], bv[hsl]], axis=1)),
            "kbias": np.ascontiguousarray(kb.reshape(32, 128).T),
            "trimask": tri, "ident2": id2,
        })
    return in_maps


def _assemble(results, bo):
    out = np.zeros((B, S, D), np.float32)
    for c in range(NCORES):
        r = results[c]
        dens = r["dens"]
        part = (r["po0"].astype(np.float32) / dens[0][:, None]
                + r["po1"].astype(np.float32) / dens[1][:, None])
        out[c // 4] += part
    out += np.asarray(bo, np.float32)
    return out


def kernel(**inputs) -> np.ndarray:
    nc = _build()
    in_maps = _host_inputs(**inputs)
    last_err = None
    for attempt in range(3):
        try:
            res = bass_utils.run_bass_kernel_spmd(
                nc, in_maps, core_ids=list(range(NCORES)))
            out = _assemble(res.results, inputs["bo"])
        except Exception as e:  # transient NRT/axon device errors
            last_err = e
            continue
        if np.isfinite(out).all():
            return out
        last_err = RuntimeError("non-finite output")
    raise last_err


def run_traced(inputs, **kwargs):
    """test.py helper: run with NTFF tracing, return (out, BassKernelResults)."""
    nc = _build()
    in_maps = _host_inputs(**inputs)
    res = bass_utils.run_bass_kernel_spmd(
        nc, in_maps, core_ids=list(range(NCORES)), trace=True, **kwargs)
    return _assemble(res.results, inputs["bo"]), res
